# revision 13
# baseline (speedup 1.0000x reference)
"""CGCNN message-passing kernel for 8 Trainium2 NeuronCores.

Strategy (per core, per layer):
  - Nodes are sharded contiguously across cores (padded so the shard size is a
    multiple of lcm(112,128)=896). h lives feature-major ([128 feat, NPC nodes])
    in SBUF for the whole kernel.
  - Edges are sharded by dst shard, grouped into 112-node dst windows, and
    padded to 128-edge chunks on the host. All index-derived structures
    (onehots, gather indices, edge attrs) are precomputed on the host into
    dense input tensors; the device kernel is pure dense compute.
  - dst-side projections + edge-attr term: PE matmul with a host-built
    combined stationary [onehot(112); ea^T(8); 0(8)] against
    [Td_window; Wc; 0] — no gather needed (dst is shard-local).
  - src-side projections: one AllGather of the per-shard projection table
    T_src = h @ [Wf_b|Ws_b], then dma_gather (SWDGE) of 1KB rows per edge.
  - segment-sum: PE matmul m^T @ onehot accumulating in PSUM per window.
  - BatchNorm: feature-major reductions + a tiny AllReduce of [sum, sumsq].
  - Pooling: PE onehot matmul per node tile + AllReduce, then the small MLP
    replicated on every core.
"""

import math
import numpy as np

import concourse.bacc as bacc
import concourse.bass as bass
import concourse.tile as tile
import concourse.mybir as mybir
from concourse.bass_utils import run_bass_kernel_spmd
from concourse.masks import make_identity

FP32 = mybir.dt.float32
I16 = mybir.dt.int16

NCORES = 8
W = 112          # dst window (nodes); onehot rows 0..111, ea rows 112..119
SPLIT = 32768    # int16 dma_gather index limit -> lo/hi table split
CALLCAP = 8      # max 128-edge chunks per dma_gather call
EPS = 1e-5


class Cfg:
    def __init__(self, N, E, G, NF=16, EF=8, H=128, L=4):
        self.N, self.E, self.G, self.NF, self.EF, self.H, self.L = N, E, G, NF, EF, H, L
        per = math.ceil(N / NCORES)
        self.NPC = math.ceil(per / 896) * 896      # shard size (mult of 112 & 128)
        self.NP = self.NPC * NCORES                # padded node count
        self.NW = self.NPC // W                    # windows per shard
        self.NT = self.NPC // 128                  # 128-node tiles per shard
        self.GP = math.ceil(G / 128) * 128         # padded graph count
        self.GH = self.GP // 128                   # graph halves


def _prep(cfg, x, edge_index, edge_attr, batch):
    """Host-side preprocessing. Returns (structure, per-core input maps)."""
    N, E, G = cfg.N, cfg.E, cfg.G
    NPC, NW = cfg.NPC, cfg.NW
    src = np.asarray(edge_index[0], dtype=np.int64)
    dst = np.asarray(edge_index[1], dtype=np.int64)
    ea = np.asarray(edge_attr, dtype=np.float32)

    core = dst // NPC
    win = (dst % NPC) // W
    half = (src >= SPLIT).astype(np.int64)
    # group id per edge; sort once
    gid = (core * NW + win) * 2 + half
    order = np.argsort(gid * E + np.arange(E), kind="stable")  # stable group sort
    gid_s, src_s, dst_s, ea_s = gid[order], src[order], dst[order], ea[order]

    ngroups = NCORES * NW * 2
    cnt = np.bincount(gid_s, minlength=ngroups).reshape(NCORES, NW, 2)
    # global (uniform across cores) chunk counts per (win, half)
    nch = np.ceil(cnt.max(axis=0) / 128).astype(np.int64)  # [NW, 2]
    # call split per (win, half)
    calls = []  # list of (w, half, chunk0, nchunks) in emission order
    chunk_of = []  # (w, half, k) for each global chunk index, in order
    for w in range(NW):
        for hf in range(2):
            n = int(nch[w, hf])
            k0 = 0
            while k0 < n:
                k = min(CALLCAP, n - k0)
                calls.append((w, hf, k0, k))
                k0 += k
            for k in range(n):
                chunk_of.append((w, hf, k))
    totch = len(chunk_of)

    group_start = np.zeros(ngroups + 1, dtype=np.int64)
    np.cumsum(np.bincount(gid_s, minlength=ngroups), out=group_start[1:])

    in_maps = []
    EFp = cfg.EF
    for c in range(NCORES):
        comb = np.zeros((totch, 128, 128), np.float32)
        oht = np.zeros((totch, 128, W), np.float32)
        sidx_cols = []
        for gi, (w, hf, k) in enumerate(chunk_of):
            g = (c * NW + w) * 2 + hf
            s0, s1 = group_start[g], group_start[g + 1]
            e0 = s0 + k * 128
            e1 = min(s1, e0 + 128)
            if e1 > e0:
                n_e = e1 - e0
                dl = (dst_s[e0:e1] - (c * NPC + w * W)).astype(np.int64)
                ee = np.arange(n_e)
                comb[gi, dl, ee] = 1.0
                comb[gi, W : W + EFp, :n_e] = ea_s[e0:e1].T
                oht[gi, ee, dl] = 1.0
        # gather indices per call
        for (w, hf, k0, k) in calls:
            g = (c * NW + w) * 2 + hf
            s0, s1 = group_start[g], group_start[g + 1]
            nidx = k * 128
            iv = np.zeros(nidx, np.int64)
            e0 = s0 + k0 * 128
            e1 = min(s1, e0 + nidx)
            if e1 > e0:
                iv[: e1 - e0] = src_s[e0:e1] - hf * SPLIT
            sidx_cols.append(iv.reshape(nidx // 16, 16).T.astype(np.int16))
        sidx = np.tile(np.concatenate(sidx_cols, axis=1), (8, 1))

        # node features, transposed + sharded
        xt = np.zeros((cfg.NF, NPC), np.float32)
        lo, hi = c * NPC, min((c + 1) * NPC, N)
        if hi > lo:
            xt[:, : hi - lo] = np.asarray(x[lo:hi], np.float32).T

        # pooling onehot [NT, 128, GP] and tail mask [128, 896]
        ohg = np.zeros((cfg.NT, 128, cfg.GP), np.float32)
        if hi > lo:
            nn = np.arange(lo, hi)
            b = np.asarray(batch[lo:hi], dtype=np.int64)
            ohg[(nn - lo) // 128, (nn - lo) % 128, b] = 1.0
        tmask = np.zeros((128, 896), np.float32)
        nreal = max(0, min(NPC, N - c * NPC))
        k = max(0, nreal - (NPC - 896))
        tmask[:, :k] = 1.0

        in_maps.append(
            {"comb": comb, "oht": oht, "sidx": sidx, "xt": xt, "ohg": ohg,
             "tmask": tmask}
        )

    struct = {"nch": nch, "calls": calls, "chunk_of": chunk_of, "totch": totch,
              "sidx_cols": sum(cl[3] * 128 // 16 for cl in calls)}
    return struct, in_maps


def _prep_weights(cfg, W_emb, b_emb, Wf, bf, Ws, bs, gamma, beta, W1, b1, W2, b2,
                  W_out, b_out, batch):
    """Replicated weight tensors, packed for the device layouts."""
    L, H, EF, G = cfg.L, cfg.H, cfg.EF, cfg.G
    f32 = np.float32
    wsrc = np.stack([np.concatenate([Wf[l][H : 2 * H], Ws[l][H : 2 * H]], 1)
                     for l in range(L)]).astype(f32)          # [L,128,256]
    wdst = np.stack([np.concatenate([Wf[l][:H], Ws[l][:H]], 1)
                     for l in range(L)]).astype(f32)          # [L,128,256]
    wc = np.zeros((L, 8, 2 * H), f32)
    wc[:, :EF, :H] = np.asarray(Wf, f32)[:, 2 * H :, :]
    wc[:, :EF, H:] = np.asarray(Ws, f32)[:, 2 * H :, :]       # [L,8,256]
    bias_b = np.zeros((L, 128, 2 * H), f32)
    bias_b[:, :W, :H] = np.asarray(bf, f32)[:, None, :]
    bias_b[:, :W, H:] = np.asarray(bs, f32)[:, None, :]       # [L,128,256]
    gb = np.zeros((L, 128, 2), f32)
    gb[:, :H, 0] = np.asarray(gamma, f32)
    gb[:, :H, 1] = np.asarray(beta, f32)
    cnt = np.bincount(np.asarray(batch, np.int64), minlength=G).astype(f32)
    invc = np.zeros((128, cfg.GH), f32)
    ic = 1.0 / np.maximum(cnt, 1.0)
    icp = np.zeros(cfg.GP, f32)
    icp[:G] = ic
    invc[:, :] = icp.reshape(cfg.GH, 128).T
    return {
        "wemb": np.asarray(W_emb, f32),                        # [NF,128]
        "bemb": np.asarray(b_emb, f32).reshape(cfg.H, 1),
        "wsrc": wsrc, "wdst": wdst, "wc": wc, "bias_b": bias_b, "gb": gb,
        "invc": invc,
        "w1": np.asarray(W1, f32), "b1": np.asarray(b1, f32).reshape(-1, 1),
        "w2": np.asarray(W2, f32), "b2": np.asarray(b2, f32).reshape(-1, 1),
        "wout": np.asarray(W_out, f32), "bout": np.asarray(b_out, f32).reshape(1, 1),
    }


def _build(cfg, struct):
    """Trace the bass program. Returns nc."""
    NPC, NW, NT, L, H, NF = cfg.NPC, cfg.NW, cfg.NT, cfg.L, cfg.H, cfg.NF
    GP, GH = cfg.GP, cfg.GH
    nch, calls, chunk_of, totch = (struct["nch"], struct["calls"],
                                   struct["chunk_of"], struct["totch"])
    AF = mybir.ActivationFunctionType

    nc = bacc.Bacc("TRN2", target_bir_lowering=False, debug=False,
                   num_devices=NCORES)

    # ---- kernel I/O ----
    d_comb = nc.declare_dram_parameter("comb", [totch, 128, 128], FP32, isOutput=False)
    d_oht = nc.declare_dram_parameter("oht", [totch, 128, W], FP32, isOutput=False)
    d_sidx = nc.declare_dram_parameter("sidx", [128, struct["sidx_cols"]], I16, isOutput=False)
    d_xt = nc.declare_dram_parameter("xt", [NF, NPC], FP32, isOutput=False)
    d_ohg = nc.declare_dram_parameter("ohg", [NT, 128, GP], FP32, isOutput=False)
    d_tmask = nc.declare_dram_parameter("tmask", [128, 896], FP32, isOutput=False)
    d_wemb = nc.declare_dram_parameter("wemb", [NF, H], FP32, isOutput=False)
    d_bemb = nc.declare_dram_parameter("bemb", [H, 1], FP32, isOutput=False)
    d_wsrc = nc.declare_dram_parameter("wsrc", [L, H, 2 * H], FP32, isOutput=False)
    d_wdst = nc.declare_dram_parameter("wdst", [L, H, 2 * H], FP32, isOutput=False)
    d_wc = nc.declare_dram_parameter("wc", [L, 8, 2 * H], FP32, isOutput=False)
    d_biasb = nc.declare_dram_parameter("bias_b", [L, 128, 2 * H], FP32, isOutput=False)
    d_gb = nc.declare_dram_parameter("gb", [L, 128, 2], FP32, isOutput=False)
    d_invc = nc.declare_dram_parameter("invc", [128, GH], FP32, isOutput=False)
    d_w1 = nc.declare_dram_parameter("w1", [H, 64], FP32, isOutput=False)
    d_b1 = nc.declare_dram_parameter("b1", [64, 1], FP32, isOutput=False)
    d_w2 = nc.declare_dram_parameter("w2", [64, 32], FP32, isOutput=False)
    d_b2 = nc.declare_dram_parameter("b2", [32, 1], FP32, isOutput=False)
    d_wout = nc.declare_dram_parameter("wout", [32, 1], FP32, isOutput=False)
    d_bout = nc.declare_dram_parameter("bout", [1, 1], FP32, isOutput=False)
    d_out = nc.declare_dram_parameter("out", [GP], FP32, isOutput=True)

    # ---- internal DRAM (collectives) ----
    ag_in = [nc.dram_tensor(f"ag_in{l}", [NPC, 2 * H], FP32) for l in range(L)]
    ts_full = [nc.dram_tensor(f"ts_full{l}", [cfg.NP, 2 * H], FP32,
                              addr_space="Shared") for l in range(L)]
    ar_in = [nc.dram_tensor(f"ar_in{l}", [128, 2], FP32) for l in range(L)]
    ar_out = [nc.dram_tensor(f"ar_out{l}", [128, 2], FP32, addr_space="Shared")
              for l in range(L)]
    pool_in = nc.dram_tensor("pool_in", [GP, H], FP32)
    pool_out = nc.dram_tensor("pool_out", [GP, H], FP32, addr_space="Shared")
    rg = [list(range(NCORES))]

    with tile.TileContext(nc) as tc:
        with (
            tc.tile_pool(name="persist", bufs=1) as pp,
            tc.tile_pool(name="lw", bufs=2) as lwp,       # per-layer weights
            tc.tile_pool(name="chunk", bufs=4) as chp,    # comb/oht tiles
            tc.tile_pool(name="gath", bufs=3) as gap,     # gather outputs
            tc.tile_pool(name="sidxp", bufs=3) as sxp,    # gather index tiles
            tc.tile_pool(name="edge", bufs=4) as edp,     # z/s1/s2/m tiles
            tc.tile_pool(name="tsout", bufs=3) as tso,    # T_src build staging
            tc.tile_pool(name="small", bufs=2) as smp,
            tc.tile_pool(name="psz", bufs=3, space="PSUM") as psz,
            tc.tile_pool(name="psagg", bufs=2, space="PSUM") as psagg,
            tc.tile_pool(name="psb", bufs=2, space="PSUM") as psb,
        ):
            h_T = pp.tile([128, NPC], FP32)
            xn_T = pp.tile([128, NPC], FP32)
            ident = pp.tile([128, 128], FP32)
            tmask_t = pp.tile([128, 896], FP32)

            nc.sync.dma_start(out=tmask_t[:], in_=d_tmask[:])
            make_identity(nc, ident[:])
            epsc = pp.tile([128, 1], FP32)
            nc.vector.memset(epsc[:], EPS)

            # ---------- embedding: h = relu(x @ W_emb + b_emb) ----------
            wemb_t = smp.tile([NF, H], FP32)
            bemb_t = pp.tile([H, 1], FP32)
            nc.sync.dma_start(out=wemb_t[:], in_=d_wemb[:])
            nc.sync.dma_start(out=bemb_t[:], in_=d_bemb[:])
            EMBW = 448
            for s in range(NPC // EMBW):
                xs_t = edp.tile([NF, EMBW], FP32, tag="xs")
                nc.sync.dma_start(out=xs_t[:],
                                  in_=d_xt[:, s * EMBW : (s + 1) * EMBW])
                ps = psb.tile([128, EMBW], FP32, space="PSUM", tag="ps")
                nc.tensor.matmul(out=ps[:], lhsT=wemb_t[:], rhs=xs_t[:],
                                 start=True, stop=True)
                nc.scalar.activation(out=h_T[:, s * EMBW : (s + 1) * EMBW],
                                     in_=ps[:], func=AF.Relu, bias=bemb_t[:, 0:1])

            # ---------- layers ----------
            for l in range(L):
                wsrc_t = lwp.tile([H, 2 * H], FP32, tag="wsrc")
                wdst_t = lwp.tile([H, 2 * H], FP32, tag="wdst")
                wc_t = lwp.tile([8, 2 * H], FP32, tag="wc")
                biasb_t = lwp.tile([128, 2 * H], FP32, tag="biasb")
                gb_t = lwp.tile([128, 2], FP32, tag="gb")
                nc.sync.dma_start(out=wsrc_t[:], in_=d_wsrc[l])
                nc.sync.dma_start(out=wdst_t[:], in_=d_wdst[l])
                nc.sync.dma_start(out=wc_t[:], in_=d_wc[l])
                nc.sync.dma_start(out=biasb_t[:], in_=d_biasb[l])
                nc.sync.dma_start(out=gb_t[:], in_=d_gb[l])

                # ---- T_src shard build + AllGather ----
                for t in range(NT):
                    ps = psb.tile([128, 2 * H], FP32, space="PSUM", tag="ps")
                    nc.tensor.matmul(out=ps[:], lhsT=h_T[:, t * 128 : (t + 1) * 128],
                                     rhs=wsrc_t[:], start=True, stop=True)
                    st = tso.tile([128, 2 * H], FP32)
                    nc.vector.tensor_copy(out=st[:], in_=ps[:])
                    nc.gpsimd.dma_start(out=ag_in[l][t * 128 : (t + 1) * 128, :],
                                        in_=st[:])
                nc.gpsimd.collective_compute(
                    "AllGather", mybir.AluOpType.bypass, replica_groups=rg,
                    ins=[ag_in[l][:]], outs=[ts_full[l][:]])

                # ---- per-window: Td build, gathers, edge chunks ----
                calls_by_w = {}
                for cl in calls:
                    calls_by_w.setdefault(cl[0], []).append(cl)
                chunks_by_w = {}
                for gi, (w, hf, k) in enumerate(chunk_of):
                    chunks_by_w.setdefault(w, []).append((gi, hf, k))
                scol = 0
                for w in range(NW):
                    wchunks = chunks_by_w.get(w, [])
                    if not wchunks:
                        nc.vector.tensor_copy(out=xn_T[:, w * W : (w + 1) * W],
                                              in_=h_T[:, w * W : (w + 1) * W])
                        continue
                    # rhs_cat for this window: [Td+bias(112); Wc(8); junk(8)]
                    ps = psb.tile([128, 2 * H], FP32, space="PSUM", tag="ps")
                    nc.tensor.matmul(out=ps[:W, :], lhsT=h_T[:, w * W : (w + 1) * W],
                                     rhs=wdst_t[:], start=True, stop=True)
                    rc = lwp.tile([128, 2 * H], FP32, tag="rc")
                    nc.vector.tensor_add(out=rc[0:W, :], in0=ps[:W, :],
                                         in1=biasb_t[0:W, :])
                    nc.sync.dma_start(out=rc[W : W + 8, :], in_=d_wc[l])
                    # src gathers for this window
                    gtiles = {}
                    for (_w, hf, k0, k) in calls_by_w.get(w, []):
                        nidx = k * 128
                        sx = sxp.tile([128, CALLCAP * 128 // 16], I16, tag="sx")
                        nc.sync.dma_start(out=sx[:, : nidx // 16],
                                          in_=d_sidx[:, scol : scol + nidx // 16])
                        scol += nidx // 16
                        g = gap.tile([128, CALLCAP, 2 * H], FP32, tag="gt")
                        base = ts_full[l][0 : min(SPLIT, cfg.NP), :] if hf == 0 \
                            else ts_full[l][SPLIT : cfg.NP, :]
                        nc.gpsimd.dma_gather(
                            out_ap=g[:, :k, :], in_ap=base,
                            idxs_ap=sx[:, : nidx // 16],
                            num_idxs=nidx, num_idxs_reg=nidx, elem_size=2 * H,
                            single_packet=False)
                        for kk in range(k):
                            gtiles[(hf, k0 + kk)] = (g, kk)
                    # edge chunks
                    agg_ps = psagg.tile([128, W], FP32, space="PSUM", tag="agg")
                    for ci, (gi, hf, k) in enumerate(wchunks):
                        comb_t = chp.tile([128, 128], FP32, tag="comb")
                        oht_t = chp.tile([128, W], FP32, tag="oht")
                        nc.sync.dma_start(out=comb_t[:], in_=d_comb[gi])
                        nc.sync.dma_start(out=oht_t[:], in_=d_oht[gi])
                        pz = psz.tile([128, 2 * H], FP32, space="PSUM", tag="pz")
                        nc.tensor.matmul(out=pz[:, :H], lhsT=comb_t[:],
                                         rhs=rc[:, :H], start=True, stop=True)
                        nc.tensor.matmul(out=pz[:, H:], lhsT=comb_t[:],
                                         rhs=rc[:, H:], start=True, stop=True)
                        g, kk = gtiles[(hf, k)]
                        z = edp.tile([128, 2 * H], FP32, tag="z")
                        nc.vector.tensor_add(out=z[:], in0=pz[:], in1=g[:, kk, :])
                        # sigma(a) = 1/(1+exp(-a)); softplus(b) = ln(exp(b)+1)
                        # (only Exp/Ln/Relu/Identity anywhere -> one ACT table)
                        s1 = edp.tile([128, H], FP32, tag="s1")
                        s2 = edp.tile([128, H], FP32, tag="s2")
                        nc.scalar.activation(out=s1[:], in_=z[:, :H], func=AF.Exp,
                                             scale=-1.0)
                        nc.vector.tensor_scalar_add(out=s1[:], in0=s1[:],
                                                    scalar1=1.0)
                        nc.vector.reciprocal(out=s1[:], in_=s1[:])
                        nc.scalar.activation(out=s2[:], in_=z[:, H:], func=AF.Exp)
                        nc.scalar.activation(out=s2[:], in_=s2[:], func=AF.Ln,
                                             bias=1.0)
                        m = edp.tile([128, H], FP32, tag="m")
                        nc.vector.tensor_mul(out=m[:], in0=s1[:], in1=s2[:])
                        nc.tensor.matmul(out=agg_ps[:], lhsT=m[:], rhs=oht_t[:],
                                         start=(ci == 0),
                                         stop=(ci == len(wchunks) - 1),
                                         skip_group_check=True)
                    nc.vector.tensor_add(out=xn_T[:, w * W : (w + 1) * W],
                                         in0=agg_ps[:],
                                         in1=h_T[:, w * W : (w + 1) * W])

                # ---- BatchNorm stats + AllReduce ----
                NB = NPC // 896
                part = smp.tile([128, 32], FP32, tag="part")
                for b in range(NB):
                    blk = xn_T[:, b * 896 : (b + 1) * 896]
                    nc.vector.reduce_sum(out=part[:, b : b + 1], in_=blk,
                                         axis=mybir.AxisListType.X)
                    sqb = edp.tile([128, 896], FP32, tag="sqb")
                    nc.vector.tensor_mul(out=sqb[:], in0=blk, in1=blk)
                    nc.vector.reduce_sum(out=part[:, 16 + b : 17 + b], in_=sqb[:],
                                         axis=mybir.AxisListType.X)
                stats = smp.tile([128, 2], FP32, tag="stats")
                nc.vector.reduce_sum(out=stats[:, 0:1], in_=part[:, 0:NB],
                                     axis=mybir.AxisListType.X)
                nc.vector.reduce_sum(out=stats[:, 1:2], in_=part[:, 16 : 16 + NB],
                                     axis=mybir.AxisListType.X)
                nc.gpsimd.dma_start(out=ar_in[l][:], in_=stats[:])
                nc.gpsimd.collective_compute(
                    "AllReduce", mybir.AluOpType.add, replica_groups=rg,
                    ins=[ar_in[l][:]], outs=[ar_out[l][:]])
                st2 = smp.tile([128, 2], FP32, tag="st2")
                nc.gpsimd.dma_start(out=st2[:], in_=ar_out[l][:])
                mu = smp.tile([128, 1], FP32, tag="mu")
                var = smp.tile([128, 1], FP32, tag="var")
                inv = smp.tile([128, 1], FP32, tag="inv")
                sc = smp.tile([128, 1], FP32, tag="sc")
                bi = smp.tile([128, 1], FP32, tag="bi")
                rN = 1.0 / cfg.N
                nc.vector.tensor_scalar_mul(out=mu[:], in0=st2[:, 0:1], scalar1=rN)
                nc.vector.tensor_scalar_mul(out=var[:], in0=st2[:, 1:2], scalar1=rN)
                nc.vector.tensor_mul(out=inv[:], in0=mu[:], in1=mu[:])
                nc.vector.tensor_sub(out=var[:], in0=var[:], in1=inv[:])
                # 1/sqrt(var+eps) = exp(-0.5*ln(var+eps)) -- stays on Exp/Ln table
                nc.scalar.activation(out=inv[:], in_=var[:], func=AF.Ln, bias=epsc[:, 0:1])
                nc.scalar.activation(out=inv[:], in_=inv[:], func=AF.Exp, scale=-0.5)
                nc.vector.tensor_mul(out=sc[:], in0=gb_t[:, 0:1], in1=inv[:])
                nc.vector.tensor_mul(out=bi[:], in0=mu[:], in1=sc[:])
                nc.vector.tensor_sub(out=bi[:], in0=gb_t[:, 1:2], in1=bi[:])
                # relu pass + masked residual update (blockwise)
                for b in range(NB):
                    blk = slice(b * 896, (b + 1) * 896)
                    rt = edp.tile([128, 896], FP32, tag="sqb")
                    nc.scalar.activation(out=rt[:], in_=xn_T[:, blk], func=AF.Relu,
                                         scale=sc[:, 0:1], bias=bi[:, 0:1])
                    if b == NB - 1:
                        nc.vector.tensor_mul(out=rt[:], in0=rt[:], in1=tmask_t[:])
                    nc.vector.tensor_add(out=h_T[:, blk], in0=h_T[:, blk],
                                         in1=rt[:])

            # ---------- pooling ----------
            pool_ps = []
            for _gh in range(GH):
                pacc = psb.tile([128, H], FP32, space="PSUM", tag="ps", name=f"pacc{_gh}")
                pool_ps.append(pacc)
            for t in range(NT):
                ohg_t = chp.tile([128, GP], FP32, tag="ohg")
                nc.sync.dma_start(out=ohg_t[:], in_=d_ohg[t])
                tps = psz.tile([128, 128], FP32, space="PSUM", tag="pz")
                nc.tensor.transpose(out=tps[:], in_=h_T[:, t * 128 : (t + 1) * 128],
                                    identity=ident[:])
                hn = edp.tile([128, 128], FP32, tag="hn")
                nc.vector.tensor_copy(out=hn[:], in_=tps[:])
                for gh in range(GH):
                    nc.tensor.matmul(out=pool_ps[gh][:],
                                     lhsT=ohg_t[:, gh * 128 : (gh + 1) * 128],
                                     rhs=hn[:], start=(t == 0), stop=(t == NT - 1),
                                     skip_group_check=True)
            for gh in range(GH):
                pt = tso.tile([128, H], FP32, tag="pt")
                nc.vector.tensor_copy(out=pt[:], in_=pool_ps[gh][:])
                nc.gpsimd.dma_start(out=pool_in[gh * 128 : (gh + 1) * 128, :],
                                    in_=pt[:])
            nc.gpsimd.collective_compute(
                "AllReduce", mybir.AluOpType.add, replica_groups=rg,
                ins=[pool_in[:]], outs=[pool_out[:]])

            # ---------- readout MLP (replicated) ----------
            invc_t = smp.tile([128, GH], FP32, tag="invc")
            nc.sync.dma_start(out=invc_t[:], in_=d_invc[:])
            pooled_T = pp.tile([128, GP], FP32)
            for gh in range(GH):
                q = edp.tile([128, H], FP32, tag="q")
                nc.gpsimd.dma_start(out=q[:], in_=pool_out[gh * 128 : (gh + 1) * 128, :])
                nc.vector.tensor_scalar_mul(out=q[:], in0=q[:],
                                            scalar1=invc_t[:, gh : gh + 1])
                tps = psz.tile([128, 128], FP32, space="PSUM", tag="pz")
                nc.tensor.transpose(out=tps[:], in_=q[:], identity=ident[:])
                nc.vector.tensor_copy(out=pooled_T[:, gh * 128 : (gh + 1) * 128],
                                      in_=tps[:])
            w1_t = smp.tile([H, 64], FP32, tag="w1")
            w2_t = smp.tile([64, 32], FP32, tag="w2")
            wout_t = smp.tile([32, 1], FP32, tag="wout")
            b1_t = smp.tile([64, 1], FP32, tag="b1")
            b2_t = smp.tile([32, 1], FP32, tag="b2")
            bout_t = smp.tile([1, 1], FP32, tag="bout")
            for tt, dd in ((w1_t, d_w1), (w2_t, d_w2), (wout_t, d_wout),
                           (b1_t, d_b1), (b2_t, d_b2), (bout_t, d_bout)):
                nc.sync.dma_start(out=tt[:], in_=dd[:])
            g1ps = psb.tile([64, GP], FP32, space="PSUM", tag="ps")
            nc.tensor.matmul(out=g1ps[:], lhsT=w1_t[:], rhs=pooled_T[:],
                             start=True, stop=True)
            g1 = edp.tile([64, GP], FP32, tag="g1")
            nc.scalar.activation(out=g1[:], in_=g1ps[:], func=AF.Relu,
                                 bias=b1_t[:, 0:1])
            g2ps = psb.tile([32, GP], FP32, space="PSUM", tag="ps")
            nc.tensor.matmul(out=g2ps[:], lhsT=w2_t[:], rhs=g1[:],
                             start=True, stop=True)
            g2 = edp.tile([32, GP], FP32, tag="g2")
            nc.scalar.activation(out=g2[:], in_=g2ps[:], func=AF.Relu,
                                 bias=b2_t[:, 0:1])
            ops = psb.tile([1, GP], FP32, space="PSUM", tag="ps")
            nc.tensor.matmul(out=ops[:], lhsT=wout_t[:], rhs=g2[:],
                             start=True, stop=True)
            ot = edp.tile([1, GP], FP32, tag="ot")
            nc.scalar.activation(out=ot[:], in_=ops[:], func=AF.Identity,
                                 bias=bout_t[:, 0:1])
            nc.sync.dma_start(out=d_out[None, :], in_=ot[:])

    nc.finalize()
    return nc


def _run(cfg, inputs, trace=False):
    struct, in_maps = _prep(cfg, inputs["x"], inputs["edge_index"],
                            inputs["edge_attr"], inputs["batch"])
    wmap = _prep_weights(cfg, inputs["W_emb"], inputs["b_emb"], inputs["Wf"],
                         inputs["bf"], inputs["Ws"], inputs["bs"],
                         inputs["gamma"], inputs["beta"], inputs["W1"],
                         inputs["b1"], inputs["W2"], inputs["b2"],
                         inputs["W_out"], inputs["b_out"], inputs["batch"])
    for m in in_maps:
        m.update(wmap)
    nc = _build(cfg, struct)
    res = run_bass_kernel_spmd(nc, in_maps, list(range(NCORES)), trace=trace)
    out = res.results[0]["out"][: cfg.G].astype(np.float32)
    return out, res


def kernel(**inputs):
    x = np.asarray(inputs["x"])
    ei = np.asarray(inputs["edge_index"])
    batch = np.asarray(inputs["batch"])
    cfg = Cfg(N=x.shape[0], E=ei.shape[1], G=256)
    out, _ = _run(cfg, inputs)
    return out.astype(np.float32)


# revision 16
# speedup vs baseline: 1.4390x; 1.4390x over previous
"""CGCNN message-passing kernel for 8 Trainium2 NeuronCores.

Strategy (per core, per layer):
  - Nodes are sharded contiguously across cores (padded so the shard size is a
    multiple of lcm(112,128)=896). h lives feature-major ([128 feat, NPC nodes])
    in SBUF for the whole kernel.
  - Edges are sharded by dst shard, grouped into 112-node dst windows, and
    padded to 128-edge chunks on the host. All index-derived structures
    (onehots, gather indices, edge attrs) are precomputed on the host into
    dense input tensors; the device kernel is pure dense compute.
  - dst-side projections + edge-attr term: PE matmul with a host-built
    combined stationary [onehot(112); ea^T(8); 0(8)] against
    [Td_window; Wc; 0] — no gather needed (dst is shard-local).
  - src-side projections: one AllGather of the per-shard projection table
    T_src = h @ [Wf_b|Ws_b], then dma_gather (SWDGE) of 1KB rows per edge.
  - segment-sum: PE matmul m^T @ onehot accumulating in PSUM per window.
  - BatchNorm: feature-major reductions + a tiny AllReduce of [sum, sumsq].
  - Pooling: PE onehot matmul per node tile + AllReduce, then the small MLP
    replicated on every core.
"""

import math
import numpy as np

import concourse.bacc as bacc
import concourse.bass as bass
import concourse.tile as tile
import concourse.mybir as mybir
from concourse.bass_utils import run_bass_kernel_spmd
from concourse.masks import make_identity

FP32 = mybir.dt.float32
I16 = mybir.dt.int16

NCORES = 8
W = 112          # dst window (nodes); onehot rows 0..111, ea rows 112..119
SPLIT = 32768    # int16 dma_gather index limit -> lo/hi table split
CALLCAP = 8      # max 128-edge chunks per dma_gather call
EPS = 1e-5


class Cfg:
    def __init__(self, N, E, G, NF=16, EF=8, H=128, L=4):
        self.N, self.E, self.G, self.NF, self.EF, self.H, self.L = N, E, G, NF, EF, H, L
        per = math.ceil(N / NCORES)
        self.NPC = math.ceil(per / 896) * 896      # shard size (mult of 112 & 128)
        self.NP = self.NPC * NCORES                # padded node count
        self.NW = self.NPC // W                    # windows per shard
        self.NT = self.NPC // 128                  # 128-node tiles per shard
        self.GP = math.ceil(G / 128) * 128         # padded graph count
        self.GH = self.GP // 128                   # graph halves


def _prep(cfg, x, edge_index, edge_attr, batch):
    """Host-side preprocessing. Returns (structure, per-core input maps)."""
    N, E, G = cfg.N, cfg.E, cfg.G
    NPC, NW = cfg.NPC, cfg.NW
    src = np.asarray(edge_index[0], dtype=np.int64)
    dst = np.asarray(edge_index[1], dtype=np.int64)
    ea = np.asarray(edge_attr, dtype=np.float32)

    core = dst // NPC
    win = (dst % NPC) // W
    half = (src >= SPLIT).astype(np.int64)
    # group id per edge; sort once
    gid = (core * NW + win) * 2 + half
    order = np.argsort(gid * E + np.arange(E), kind="stable")  # stable group sort
    gid_s, src_s, dst_s, ea_s = gid[order], src[order], dst[order], ea[order]

    ngroups = NCORES * NW * 2
    cnt = np.bincount(gid_s, minlength=ngroups).reshape(NCORES, NW, 2)
    # global (uniform across cores) chunk counts per (win, half)
    nch = np.ceil(cnt.max(axis=0) / 128).astype(np.int64)  # [NW, 2]
    nch = ((nch + 1) // 2) * 2  # even so edge chunks can be processed in pairs
    # call split per (win, half)
    calls = []  # list of (w, half, chunk0, nchunks) in emission order
    chunk_of = []  # (w, half, k) for each global chunk index, in order
    for w in range(NW):
        for hf in range(2):
            n = int(nch[w, hf])
            k0 = 0
            while k0 < n:
                k = min(CALLCAP, n - k0)
                calls.append((w, hf, k0, k))
                k0 += k
            for k in range(n):
                chunk_of.append((w, hf, k))
    totch = len(chunk_of)

    group_start = np.zeros(ngroups + 1, dtype=np.int64)
    np.cumsum(np.bincount(gid_s, minlength=ngroups), out=group_start[1:])

    in_maps = []
    EFp = cfg.EF
    for c in range(NCORES):
        # ctab: cols 0..127 = comb ([onehot(112); ea(EF); 0]), cols 128..239 = oht
        ctab = np.zeros((totch, 128, 240), np.float32)
        sidx_cols = []
        for gi, (w, hf, k) in enumerate(chunk_of):
            g = (c * NW + w) * 2 + hf
            s0, s1 = group_start[g], group_start[g + 1]
            e0 = s0 + k * 128
            e1 = min(s1, e0 + 128)
            if e1 > e0:
                n_e = e1 - e0
                dl = (dst_s[e0:e1] - (c * NPC + w * W)).astype(np.int64)
                ee = np.arange(n_e)
                ctab[gi, dl, ee] = 1.0
                ctab[gi, W : W + EFp, :n_e] = ea_s[e0:e1].T
                ctab[gi, ee, 128 + dl] = 1.0
        # gather indices per call
        for (w, hf, k0, k) in calls:
            g = (c * NW + w) * 2 + hf
            s0, s1 = group_start[g], group_start[g + 1]
            nidx = k * 128
            iv = np.zeros(nidx, np.int64)
            e0 = s0 + k0 * 128
            e1 = min(s1, e0 + nidx)
            if e1 > e0:
                iv[: e1 - e0] = src_s[e0:e1] - hf * SPLIT
            sidx_cols.append(iv.reshape(nidx // 16, 16).T.astype(np.int16))
        sidx = np.tile(np.concatenate(sidx_cols, axis=1), (8, 1))

        # node features, transposed + sharded
        xt = np.zeros((cfg.NF, NPC), np.float32)
        lo, hi = c * NPC, min((c + 1) * NPC, N)
        if hi > lo:
            xt[:, : hi - lo] = np.asarray(x[lo:hi], np.float32).T

        # pooling onehot [NT, 128, GP] and tail mask [128, 896]
        ohg = np.zeros((cfg.NT, 128, cfg.GP), np.float32)
        if hi > lo:
            nn = np.arange(lo, hi)
            b = np.asarray(batch[lo:hi], dtype=np.int64)
            ohg[(nn - lo) // 128, (nn - lo) % 128, b] = 1.0
        tmask = np.zeros((128, 896), np.float32)
        nreal = max(0, min(NPC, N - c * NPC))
        k = max(0, nreal - (NPC - 896))
        tmask[:, :k] = 1.0

        in_maps.append(
            {"ctab": ctab, "sidx": sidx, "xt": xt, "ohg": ohg, "tmask": tmask}
        )

    struct = {"nch": nch, "calls": calls, "chunk_of": chunk_of, "totch": totch,
              "sidx_cols": sum(cl[3] * 128 // 16 for cl in calls)}
    return struct, in_maps


def _prep_weights(cfg, W_emb, b_emb, Wf, bf, Ws, bs, gamma, beta, W1, b1, W2, b2,
                  W_out, b_out, batch):
    """Replicated weight tensors, packed for the device layouts."""
    L, H, EF, G = cfg.L, cfg.H, cfg.EF, cfg.G
    f32 = np.float32
    # The f-half (gate) is NEGATED everywhere so that one Exp(z) yields
    # exp(-mf) on the f-half and exp(ms) on the s-half.
    wsrc = np.stack([np.concatenate([-Wf[l][H : 2 * H], Ws[l][H : 2 * H]], 1)
                     for l in range(L)]).astype(f32)          # [L,128,256]
    wdst = np.stack([np.concatenate([-Wf[l][:H], Ws[l][:H]], 1)
                     for l in range(L)]).astype(f32)          # [L,128,256]
    wc = np.zeros((L, 8, 2 * H), f32)
    wc[:, :EF, :H] = -np.asarray(Wf, f32)[:, 2 * H :, :]
    wc[:, :EF, H:] = np.asarray(Ws, f32)[:, 2 * H :, :]       # [L,8,256]
    bias_b = np.zeros((L, 128, 2 * H), f32)
    bias_b[:, :W, :H] = -np.asarray(bf, f32)[:, None, :]
    bias_b[:, :W, H:] = np.asarray(bs, f32)[:, None, :]       # [L,128,256]
    gb = np.zeros((L, 128, 2), f32)
    gb[:, :H, 0] = np.asarray(gamma, f32)
    gb[:, :H, 1] = np.asarray(beta, f32)
    cnt = np.bincount(np.asarray(batch, np.int64), minlength=G).astype(f32)
    invc = np.zeros((128, cfg.GH), f32)
    ic = 1.0 / np.maximum(cnt, 1.0)
    icp = np.zeros(cfg.GP, f32)
    icp[:G] = ic
    invc[:, :] = icp.reshape(cfg.GH, 128).T
    return {
        "wemb": np.asarray(W_emb, f32),                        # [NF,128]
        "bemb": np.asarray(b_emb, f32).reshape(cfg.H, 1),
        "wsrc": wsrc, "wdst": wdst, "wc": wc, "bias_b": bias_b, "gb": gb,
        "invc": invc,
        "w1": np.asarray(W1, f32), "b1": np.asarray(b1, f32).reshape(-1, 1),
        "w2": np.asarray(W2, f32), "b2": np.asarray(b2, f32).reshape(-1, 1),
        "wout": np.asarray(W_out, f32), "bout": np.asarray(b_out, f32).reshape(1, 1),
    }


def _build(cfg, struct):
    """Trace the bass program. Returns nc."""
    NPC, NW, NT, L, H, NF = cfg.NPC, cfg.NW, cfg.NT, cfg.L, cfg.H, cfg.NF
    GP, GH = cfg.GP, cfg.GH
    nch, calls, chunk_of, totch = (struct["nch"], struct["calls"],
                                   struct["chunk_of"], struct["totch"])
    AF = mybir.ActivationFunctionType

    nc = bacc.Bacc("TRN2", target_bir_lowering=False, debug=False,
                   num_devices=NCORES)

    # ---- kernel I/O ----
    d_ctab = nc.declare_dram_parameter("ctab", [totch, 128, 240], FP32, isOutput=False)
    d_sidx = nc.declare_dram_parameter("sidx", [128, struct["sidx_cols"]], I16, isOutput=False)
    d_xt = nc.declare_dram_parameter("xt", [NF, NPC], FP32, isOutput=False)
    d_ohg = nc.declare_dram_parameter("ohg", [NT, 128, GP], FP32, isOutput=False)
    d_tmask = nc.declare_dram_parameter("tmask", [128, 896], FP32, isOutput=False)
    d_wemb = nc.declare_dram_parameter("wemb", [NF, H], FP32, isOutput=False)
    d_bemb = nc.declare_dram_parameter("bemb", [H, 1], FP32, isOutput=False)
    d_wsrc = nc.declare_dram_parameter("wsrc", [L, H, 2 * H], FP32, isOutput=False)
    d_wdst = nc.declare_dram_parameter("wdst", [L, H, 2 * H], FP32, isOutput=False)
    d_wc = nc.declare_dram_parameter("wc", [L, 8, 2 * H], FP32, isOutput=False)
    d_biasb = nc.declare_dram_parameter("bias_b", [L, 128, 2 * H], FP32, isOutput=False)
    d_gb = nc.declare_dram_parameter("gb", [L, 128, 2], FP32, isOutput=False)
    d_invc = nc.declare_dram_parameter("invc", [128, GH], FP32, isOutput=False)
    d_w1 = nc.declare_dram_parameter("w1", [H, 64], FP32, isOutput=False)
    d_b1 = nc.declare_dram_parameter("b1", [64, 1], FP32, isOutput=False)
    d_w2 = nc.declare_dram_parameter("w2", [64, 32], FP32, isOutput=False)
    d_b2 = nc.declare_dram_parameter("b2", [32, 1], FP32, isOutput=False)
    d_wout = nc.declare_dram_parameter("wout", [32, 1], FP32, isOutput=False)
    d_bout = nc.declare_dram_parameter("bout", [1, 1], FP32, isOutput=False)
    d_out = nc.declare_dram_parameter("out", [GP], FP32, isOutput=True)

    # ---- internal DRAM (collectives) ----
    ag_in = [nc.dram_tensor(f"ag_in{l}", [NPC, 2 * H], FP32) for l in range(L)]
    ts_full = [nc.dram_tensor(f"ts_full{l}", [cfg.NP, 2 * H], FP32,
                              addr_space="Shared") for l in range(L)]
    ar_in = [nc.dram_tensor(f"ar_in{l}", [128, 2], FP32) for l in range(L)]
    ar_out = [nc.dram_tensor(f"ar_out{l}", [128, 2], FP32, addr_space="Shared")
              for l in range(L)]
    pool_in = nc.dram_tensor("pool_in", [GP, H], FP32)
    pool_out = nc.dram_tensor("pool_out", [GP, H], FP32, addr_space="Shared")
    rg = [list(range(NCORES))]

    with tile.TileContext(nc) as tc:
        with (
            tc.tile_pool(name="persist", bufs=1) as pp,
            tc.tile_pool(name="lw", bufs=2) as lwp,       # per-layer weights
            tc.tile_pool(name="chunk", bufs=4) as chp,    # comb/oht tiles
            tc.tile_pool(name="gath", bufs=3) as gap,     # gather outputs
            tc.tile_pool(name="sidxp", bufs=3) as sxp,    # gather index tiles
            tc.tile_pool(name="edge", bufs=4) as edp,     # z/s1/s2/m tiles
            tc.tile_pool(name="tsout", bufs=3) as tso,    # T_src build staging
            tc.tile_pool(name="small", bufs=2) as smp,
            tc.tile_pool(name="psz", bufs=3, space="PSUM") as psz,
            tc.tile_pool(name="psagg", bufs=2, space="PSUM") as psagg,
            tc.tile_pool(name="psb", bufs=2, space="PSUM") as psb,
        ):
            h_T = pp.tile([128, NPC], FP32)
            xn_T = pp.tile([128, NPC], FP32)
            ident = pp.tile([128, 128], FP32)
            tmask_t = pp.tile([128, 896], FP32)

            nc.sync.dma_start(out=tmask_t[:], in_=d_tmask[:])
            make_identity(nc, ident[:])
            epsc = pp.tile([128, 1], FP32)
            nc.vector.memset(epsc[:], EPS)

            # ---------- embedding: h = relu(x @ W_emb + b_emb) ----------
            wemb_t = smp.tile([NF, H], FP32)
            bemb_t = pp.tile([H, 1], FP32)
            nc.sync.dma_start(out=wemb_t[:], in_=d_wemb[:])
            nc.sync.dma_start(out=bemb_t[:], in_=d_bemb[:])
            EMBW = 448
            for s in range(NPC // EMBW):
                xs_t = edp.tile([NF, EMBW], FP32, tag="xs")
                nc.sync.dma_start(out=xs_t[:],
                                  in_=d_xt[:, s * EMBW : (s + 1) * EMBW])
                ps = psb.tile([128, EMBW], FP32, space="PSUM", tag="ps")
                nc.tensor.matmul(out=ps[:], lhsT=wemb_t[:], rhs=xs_t[:],
                                 start=True, stop=True)
                nc.scalar.activation(out=h_T[:, s * EMBW : (s + 1) * EMBW],
                                     in_=ps[:], func=AF.Relu, bias=bemb_t[:, 0:1])

            # ---------- layers ----------
            for l in range(L):
                wsrc_t = lwp.tile([H, 2 * H], FP32, tag="wsrc")
                wdst_t = lwp.tile([H, 2 * H], FP32, tag="wdst")
                wc_t = lwp.tile([8, 2 * H], FP32, tag="wc")
                biasb_t = lwp.tile([128, 2 * H], FP32, tag="biasb")
                gb_t = lwp.tile([128, 2], FP32, tag="gb")
                nc.sync.dma_start(out=wsrc_t[:], in_=d_wsrc[l])
                nc.sync.dma_start(out=wdst_t[:], in_=d_wdst[l])
                nc.sync.dma_start(out=wc_t[:], in_=d_wc[l])
                nc.sync.dma_start(out=biasb_t[:], in_=d_biasb[l])
                nc.sync.dma_start(out=gb_t[:], in_=d_gb[l])

                # ---- T_src shard build + AllGather ----
                for t in range(NT):
                    ps = psb.tile([128, 2 * H], FP32, space="PSUM", tag="ps")
                    nc.tensor.matmul(out=ps[:], lhsT=h_T[:, t * 128 : (t + 1) * 128],
                                     rhs=wsrc_t[:], start=True, stop=True)
                    st = tso.tile([128, 2 * H], FP32)
                    nc.vector.tensor_copy(out=st[:], in_=ps[:])
                    nc.gpsimd.dma_start(out=ag_in[l][t * 128 : (t + 1) * 128, :],
                                        in_=st[:])
                nc.gpsimd.collective_compute(
                    "AllGather", mybir.AluOpType.bypass, replica_groups=rg,
                    ins=[ag_in[l][:]], outs=[ts_full[l][:]])

                # ---- per-window: Td build, gathers, edge chunks ----
                calls_by_w = {}
                for cl in calls:
                    calls_by_w.setdefault(cl[0], []).append(cl)
                chunks_by_w = {}
                for gi, (w, hf, k) in enumerate(chunk_of):
                    chunks_by_w.setdefault(w, []).append((gi, hf, k))
                scol = 0
                for w in range(NW):
                    wchunks = chunks_by_w.get(w, [])
                    if not wchunks:
                        nc.vector.tensor_copy(out=xn_T[:, w * W : (w + 1) * W],
                                              in_=h_T[:, w * W : (w + 1) * W])
                        continue
                    # rhs_cat for this window: [Td+bias(112); Wc(8); junk(8)]
                    ps = psb.tile([128, 2 * H], FP32, space="PSUM", tag="ps")
                    nc.tensor.matmul(out=ps[:W, :], lhsT=h_T[:, w * W : (w + 1) * W],
                                     rhs=wdst_t[:], start=True, stop=True)
                    rc = lwp.tile([128, 2 * H], FP32, tag="rc")
                    nc.vector.tensor_add(out=rc[0:W, :], in0=ps[:W, :],
                                         in1=biasb_t[0:W, :])
                    nc.sync.dma_start(out=rc[W : W + 8, :], in_=d_wc[l])
                    # src gathers for this window
                    gtiles = {}
                    for (_w, hf, k0, k) in calls_by_w.get(w, []):
                        nidx = k * 128
                        sx = sxp.tile([128, CALLCAP * 128 // 16], I16, tag="sx")
                        nc.sync.dma_start(out=sx[:, : nidx // 16],
                                          in_=d_sidx[:, scol : scol + nidx // 16])
                        scol += nidx // 16
                        g = gap.tile([128, CALLCAP, 2 * H], FP32, tag="gt")
                        base = ts_full[l][0 : min(SPLIT, cfg.NP), :] if hf == 0 \
                            else ts_full[l][SPLIT : cfg.NP, :]
                        nc.gpsimd.dma_gather(
                            out_ap=g[:, :k, :], in_ap=base,
                            idxs_ap=sx[:, : nidx // 16],
                            num_idxs=nidx, num_idxs_reg=nidx, elem_size=2 * H,
                            single_packet=False)
                        for kk in range(k):
                            gtiles[(hf, k0 + kk)] = (g, kk)
                    # edge chunks, processed in pairs (even count per window)
                    agg_ps = psagg.tile([128, W], FP32, space="PSUM", tag="agg")
                    npair = len(wchunks) // 2
                    for pi in range(npair):
                        gi, hf, k = wchunks[2 * pi]
                        ct = chp.tile([128, 2, 240], FP32, tag="ct")
                        nc.sync.dma_start(
                            out=ct[:],
                            in_=d_ctab[gi : gi + 2].rearrange("c p e -> p c e"))
                        pz = psz.tile([128, 2, 2 * H], FP32, space="PSUM", tag="pz")
                        for j in range(2):
                            nc.tensor.matmul(out=pz[:, j, :],
                                             lhsT=ct[:, j, 0:128], rhs=rc[:],
                                             start=True, stop=True)
                        g, kk = gtiles[(hf, k)]
                        z = edp.tile([128, 2, 2 * H], FP32, tag="z")
                        nc.vector.tensor_add(out=z[:], in0=pz[:],
                                             in1=g[:, kk : kk + 2, :])
                        # z = [-mf | ms] per chunk; e = exp(z); then
                        # m = ln(1+e_s) / (1+e_f)  (= softplus(ms)*sigmoid(mf))
                        e = edp.tile([128, 2, 2 * H], FP32, tag="e")
                        nc.scalar.activation(out=e[:], in_=z[:], func=AF.Exp)
                        t = edp.tile([128, 2, H], FP32, tag="t")
                        nc.scalar.activation(out=t[:], in_=e[:, :, H:], func=AF.Ln,
                                             bias=1.0)
                        d = edp.tile([128, 2, H], FP32, tag="d")
                        nc.vector.tensor_scalar_add(out=d[:], in0=e[:, :, :H],
                                                    scalar1=1.0)
                        r = edp.tile([128, 2, H], FP32, tag="r")
                        nc.vector.reciprocal_approx_fast(out=r[:], in_=d[:])
                        m = edp.tile([128, 2, H], FP32, tag="m")
                        nc.vector.tensor_mul(out=m[:], in0=t[:], in1=r[:])
                        for j in range(2):
                            nc.tensor.matmul(out=agg_ps[:], lhsT=m[:, j, :],
                                             rhs=ct[:, j, 128:240],
                                             start=(pi == 0 and j == 0),
                                             stop=(pi == npair - 1 and j == 1),
                                             skip_group_check=True)
                    nc.vector.tensor_add(out=xn_T[:, w * W : (w + 1) * W],
                                         in0=agg_ps[:],
                                         in1=h_T[:, w * W : (w + 1) * W])

                # ---- BatchNorm stats + AllReduce ----
                NB = NPC // 896
                part = smp.tile([128, 32], FP32, tag="part")
                for b in range(NB):
                    blk = xn_T[:, b * 896 : (b + 1) * 896]
                    nc.vector.reduce_sum(out=part[:, b : b + 1], in_=blk,
                                         axis=mybir.AxisListType.X)
                    sqb = edp.tile([128, 896], FP32, tag="sqb")
                    nc.vector.tensor_mul(out=sqb[:], in0=blk, in1=blk)
                    nc.vector.reduce_sum(out=part[:, 16 + b : 17 + b], in_=sqb[:],
                                         axis=mybir.AxisListType.X)
                stats = smp.tile([128, 2], FP32, tag="stats")
                nc.vector.reduce_sum(out=stats[:, 0:1], in_=part[:, 0:NB],
                                     axis=mybir.AxisListType.X)
                nc.vector.reduce_sum(out=stats[:, 1:2], in_=part[:, 16 : 16 + NB],
                                     axis=mybir.AxisListType.X)
                nc.gpsimd.dma_start(out=ar_in[l][:], in_=stats[:])
                nc.gpsimd.collective_compute(
                    "AllReduce", mybir.AluOpType.add, replica_groups=rg,
                    ins=[ar_in[l][:]], outs=[ar_out[l][:]])
                st2 = smp.tile([128, 2], FP32, tag="st2")
                nc.gpsimd.dma_start(out=st2[:], in_=ar_out[l][:])
                mu = smp.tile([128, 1], FP32, tag="mu")
                var = smp.tile([128, 1], FP32, tag="var")
                inv = smp.tile([128, 1], FP32, tag="inv")
                sc = smp.tile([128, 1], FP32, tag="sc")
                bi = smp.tile([128, 1], FP32, tag="bi")
                rN = 1.0 / cfg.N
                nc.vector.tensor_scalar_mul(out=mu[:], in0=st2[:, 0:1], scalar1=rN)
                nc.vector.tensor_scalar_mul(out=var[:], in0=st2[:, 1:2], scalar1=rN)
                nc.vector.tensor_mul(out=inv[:], in0=mu[:], in1=mu[:])
                nc.vector.tensor_sub(out=var[:], in0=var[:], in1=inv[:])
                # 1/sqrt(var+eps) = exp(-0.5*ln(var+eps)) -- stays on Exp/Ln table
                nc.scalar.activation(out=inv[:], in_=var[:], func=AF.Ln, bias=epsc[:, 0:1])
                nc.scalar.activation(out=inv[:], in_=inv[:], func=AF.Exp, scale=-0.5)
                nc.vector.tensor_mul(out=sc[:], in0=gb_t[:, 0:1], in1=inv[:])
                nc.vector.tensor_mul(out=bi[:], in0=mu[:], in1=sc[:])
                nc.vector.tensor_sub(out=bi[:], in0=gb_t[:, 1:2], in1=bi[:])
                # relu pass + masked residual update (blockwise)
                for b in range(NB):
                    blk = slice(b * 896, (b + 1) * 896)
                    rt = edp.tile([128, 896], FP32, tag="sqb")
                    nc.scalar.activation(out=rt[:], in_=xn_T[:, blk], func=AF.Relu,
                                         scale=sc[:, 0:1], bias=bi[:, 0:1])
                    if b == NB - 1:
                        nc.vector.tensor_mul(out=rt[:], in0=rt[:], in1=tmask_t[:])
                    nc.vector.tensor_add(out=h_T[:, blk], in0=h_T[:, blk],
                                         in1=rt[:])

            # ---------- pooling ----------
            pool_ps = []
            for _gh in range(GH):
                pacc = psb.tile([128, H], FP32, space="PSUM", tag="ps", name=f"pacc{_gh}")
                pool_ps.append(pacc)
            for t in range(NT):
                ohg_t = chp.tile([128, GP], FP32, tag="ohg")
                nc.sync.dma_start(out=ohg_t[:], in_=d_ohg[t])
                tps = psz.tile([128, 128], FP32, space="PSUM", tag="pz")
                nc.tensor.transpose(out=tps[:], in_=h_T[:, t * 128 : (t + 1) * 128],
                                    identity=ident[:])
                hn = edp.tile([128, 128], FP32, tag="hn")
                nc.vector.tensor_copy(out=hn[:], in_=tps[:])
                for gh in range(GH):
                    nc.tensor.matmul(out=pool_ps[gh][:],
                                     lhsT=ohg_t[:, gh * 128 : (gh + 1) * 128],
                                     rhs=hn[:], start=(t == 0), stop=(t == NT - 1),
                                     skip_group_check=True)
            for gh in range(GH):
                pt = tso.tile([128, H], FP32, tag="pt")
                nc.vector.tensor_copy(out=pt[:], in_=pool_ps[gh][:])
                nc.gpsimd.dma_start(out=pool_in[gh * 128 : (gh + 1) * 128, :],
                                    in_=pt[:])
            nc.gpsimd.collective_compute(
                "AllReduce", mybir.AluOpType.add, replica_groups=rg,
                ins=[pool_in[:]], outs=[pool_out[:]])

            # ---------- readout MLP (replicated) ----------
            invc_t = smp.tile([128, GH], FP32, tag="invc")
            nc.sync.dma_start(out=invc_t[:], in_=d_invc[:])
            pooled_T = pp.tile([128, GP], FP32)
            for gh in range(GH):
                q = edp.tile([128, H], FP32, tag="q")
                nc.gpsimd.dma_start(out=q[:], in_=pool_out[gh * 128 : (gh + 1) * 128, :])
                nc.vector.tensor_scalar_mul(out=q[:], in0=q[:],
                                            scalar1=invc_t[:, gh : gh + 1])
                tps = psz.tile([128, 128], FP32, space="PSUM", tag="pz")
                nc.tensor.transpose(out=tps[:], in_=q[:], identity=ident[:])
                nc.vector.tensor_copy(out=pooled_T[:, gh * 128 : (gh + 1) * 128],
                                      in_=tps[:])
            w1_t = smp.tile([H, 64], FP32, tag="w1")
            w2_t = smp.tile([64, 32], FP32, tag="w2")
            wout_t = smp.tile([32, 1], FP32, tag="wout")
            b1_t = smp.tile([64, 1], FP32, tag="b1")
            b2_t = smp.tile([32, 1], FP32, tag="b2")
            bout_t = smp.tile([1, 1], FP32, tag="bout")
            for tt, dd in ((w1_t, d_w1), (w2_t, d_w2), (wout_t, d_wout),
                           (b1_t, d_b1), (b2_t, d_b2), (bout_t, d_bout)):
                nc.sync.dma_start(out=tt[:], in_=dd[:])
            g1ps = psb.tile([64, GP], FP32, space="PSUM", tag="ps")
            nc.tensor.matmul(out=g1ps[:], lhsT=w1_t[:], rhs=pooled_T[:],
                             start=True, stop=True)
            g1 = edp.tile([64, GP], FP32, tag="g1")
            nc.scalar.activation(out=g1[:], in_=g1ps[:], func=AF.Relu,
                                 bias=b1_t[:, 0:1])
            g2ps = psb.tile([32, GP], FP32, space="PSUM", tag="ps")
            nc.tensor.matmul(out=g2ps[:], lhsT=w2_t[:], rhs=g1[:],
                             start=True, stop=True)
            g2 = edp.tile([32, GP], FP32, tag="g2")
            nc.scalar.activation(out=g2[:], in_=g2ps[:], func=AF.Relu,
                                 bias=b2_t[:, 0:1])
            ops = psb.tile([1, GP], FP32, space="PSUM", tag="ps")
            nc.tensor.matmul(out=ops[:], lhsT=wout_t[:], rhs=g2[:],
                             start=True, stop=True)
            ot = edp.tile([1, GP], FP32, tag="ot")
            nc.scalar.activation(out=ot[:], in_=ops[:], func=AF.Identity,
                                 bias=bout_t[:, 0:1])
            nc.sync.dma_start(out=d_out[None, :], in_=ot[:])

    # Pin all activations to the one table containing {exp, ln, relu,
    # identity, copy} so no ACT_TABLE_LOAD thrash occurs between Exp and Ln.
    import concourse.bacc as _bacc_mod
    _orig_gat = _bacc_mod.get_activation_tables

    def _pinned(arch):
        tabs = _orig_gat(arch)
        keep = "natural_log_exp_and_others"
        assert keep in tabs
        return {k: (v if k == keep else set()) for k, v in tabs.items()}

    _bacc_mod.get_activation_tables = _pinned
    try:
        nc.finalize()
    finally:
        _bacc_mod.get_activation_tables = _orig_gat
    return nc


def _run(cfg, inputs, trace=False):
    struct, in_maps = _prep(cfg, inputs["x"], inputs["edge_index"],
                            inputs["edge_attr"], inputs["batch"])
    wmap = _prep_weights(cfg, inputs["W_emb"], inputs["b_emb"], inputs["Wf"],
                         inputs["bf"], inputs["Ws"], inputs["bs"],
                         inputs["gamma"], inputs["beta"], inputs["W1"],
                         inputs["b1"], inputs["W2"], inputs["b2"],
                         inputs["W_out"], inputs["b_out"], inputs["batch"])
    for m in in_maps:
        m.update(wmap)
    nc = _build(cfg, struct)
    res = run_bass_kernel_spmd(nc, in_maps, list(range(NCORES)), trace=trace)
    out = res.results[0]["out"][: cfg.G].astype(np.float32)
    return out, res


def kernel(**inputs):
    x = np.asarray(inputs["x"])
    ei = np.asarray(inputs["edge_index"])
    batch = np.asarray(inputs["batch"])
    cfg = Cfg(N=x.shape[0], E=ei.shape[1], G=256)
    out, _ = _run(cfg, inputs)
    return out.astype(np.float32)


# revision 17
# speedup vs baseline: 1.9078x; 1.3258x over previous
"""CGCNN message-passing kernel for 8 Trainium2 NeuronCores.

Strategy (per core, per layer):
  - Nodes are sharded contiguously across cores (padded so the shard size is a
    multiple of lcm(112,128)=896). h lives feature-major ([128 feat, NPC nodes])
    in SBUF for the whole kernel.
  - Edges are sharded by dst shard, grouped into 112-node dst windows, and
    padded to 128-edge chunks on the host. All index-derived structures
    (onehots, gather indices, edge attrs) are precomputed on the host into
    dense input tensors; the device kernel is pure dense compute.
  - dst-side projections + edge-attr term: PE matmul with a host-built
    combined stationary [onehot(112); ea^T(8); 0(8)] against
    [Td_window; Wc; 0] — no gather needed (dst is shard-local).
  - src-side projections: one AllGather of the per-shard projection table
    T_src = h @ [Wf_b|Ws_b], then dma_gather (SWDGE) of 1KB rows per edge.
  - segment-sum: PE matmul m^T @ onehot accumulating in PSUM per window.
  - BatchNorm: feature-major reductions + a tiny AllReduce of [sum, sumsq].
  - Pooling: PE onehot matmul per node tile + AllReduce, then the small MLP
    replicated on every core.
"""

import math
import numpy as np

import concourse.bacc as bacc
import concourse.bass as bass
import concourse.tile as tile
import concourse.mybir as mybir
from concourse.bass_utils import run_bass_kernel_spmd
from concourse.masks import make_identity

FP32 = mybir.dt.float32
I16 = mybir.dt.int16

NCORES = 8
W = 112          # dst window (nodes); onehot rows 0..111, ea rows 112..119
SPLIT = 32768    # int16 dma_gather index limit -> lo/hi table split
CALLCAP = 4      # max 128-edge chunks per dma_gather call
EPS = 1e-5


class Cfg:
    def __init__(self, N, E, G, NF=16, EF=8, H=128, L=4):
        self.N, self.E, self.G, self.NF, self.EF, self.H, self.L = N, E, G, NF, EF, H, L
        per = math.ceil(N / NCORES)
        self.NPC = math.ceil(per / 896) * 896      # shard size (mult of 112 & 128)
        self.NP = self.NPC * NCORES                # padded node count
        self.NW = self.NPC // W                    # windows per shard
        self.NT = self.NPC // 128                  # 128-node tiles per shard
        self.GP = math.ceil(G / 128) * 128         # padded graph count
        self.GH = self.GP // 128                   # graph halves


def _prep(cfg, x, edge_index, edge_attr, batch):
    """Host-side preprocessing. Returns (structure, per-core input maps)."""
    N, E, G = cfg.N, cfg.E, cfg.G
    NPC, NW = cfg.NPC, cfg.NW
    src = np.asarray(edge_index[0], dtype=np.int64)
    dst = np.asarray(edge_index[1], dtype=np.int64)
    ea = np.asarray(edge_attr, dtype=np.float32)

    core = dst // NPC
    win = (dst % NPC) // W
    half = (src >= SPLIT).astype(np.int64)
    # group id per edge; sort once
    gid = (core * NW + win) * 2 + half
    order = np.argsort(gid * E + np.arange(E), kind="stable")  # stable group sort
    gid_s, src_s, dst_s, ea_s = gid[order], src[order], dst[order], ea[order]

    ngroups = NCORES * NW * 2
    cnt = np.bincount(gid_s, minlength=ngroups).reshape(NCORES, NW, 2)
    # global (uniform across cores) chunk counts per (win, half)
    nch = np.ceil(cnt.max(axis=0) / 128).astype(np.int64)  # [NW, 2]
    nch = ((nch + 1) // 2) * 2  # even so edge chunks can be processed in pairs
    # call split per (win, half)
    calls = []  # list of (w, half, chunk0, nchunks) in emission order
    chunk_of = []  # (w, half, k) for each global chunk index, in order
    for w in range(NW):
        for hf in range(2):
            n = int(nch[w, hf])
            k0 = 0
            while k0 < n:
                k = min(CALLCAP, n - k0)
                calls.append((w, hf, k0, k))
                k0 += k
            for k in range(n):
                chunk_of.append((w, hf, k))
    totch = len(chunk_of)

    group_start = np.zeros(ngroups + 1, dtype=np.int64)
    np.cumsum(np.bincount(gid_s, minlength=ngroups), out=group_start[1:])

    in_maps = []
    EFp = cfg.EF
    for c in range(NCORES):
        # ctab: cols 0..127 = comb ([onehot(112); ea(EF); 0]), cols 128..239 = oht
        ctab = np.zeros((totch, 128, 240), np.float32)
        sidx_cols = []
        for gi, (w, hf, k) in enumerate(chunk_of):
            g = (c * NW + w) * 2 + hf
            s0, s1 = group_start[g], group_start[g + 1]
            e0 = s0 + k * 128
            e1 = min(s1, e0 + 128)
            if e1 > e0:
                n_e = e1 - e0
                dl = (dst_s[e0:e1] - (c * NPC + w * W)).astype(np.int64)
                ee = np.arange(n_e)
                ctab[gi, dl, ee] = 1.0
                ctab[gi, W : W + EFp, :n_e] = ea_s[e0:e1].T
                ctab[gi, ee, 128 + dl] = 1.0
        # gather indices per call
        for (w, hf, k0, k) in calls:
            g = (c * NW + w) * 2 + hf
            s0, s1 = group_start[g], group_start[g + 1]
            nidx = k * 128
            iv = np.zeros(nidx, np.int64)
            e0 = s0 + k0 * 128
            e1 = min(s1, e0 + nidx)
            if e1 > e0:
                iv[: e1 - e0] = src_s[e0:e1] - hf * SPLIT
            sidx_cols.append(iv.reshape(nidx // 16, 16).T.astype(np.int16))
        sidx = np.tile(np.concatenate(sidx_cols, axis=1), (8, 1))

        # node features, transposed + sharded
        xt = np.zeros((cfg.NF, NPC), np.float32)
        lo, hi = c * NPC, min((c + 1) * NPC, N)
        if hi > lo:
            xt[:, : hi - lo] = np.asarray(x[lo:hi], np.float32).T

        # pooling onehot [NT, 128, GP] and tail mask [128, 896]
        ohg = np.zeros((cfg.NT, 128, cfg.GP), np.float32)
        if hi > lo:
            nn = np.arange(lo, hi)
            b = np.asarray(batch[lo:hi], dtype=np.int64)
            ohg[(nn - lo) // 128, (nn - lo) % 128, b] = 1.0
        tmask = np.zeros((128, 896), np.float32)
        nreal = max(0, min(NPC, N - c * NPC))
        k = max(0, nreal - (NPC - 896))
        tmask[:, :k] = 1.0

        in_maps.append(
            {"ctab": ctab, "sidx": sidx, "xt": xt, "ohg": ohg, "tmask": tmask}
        )

    struct = {"nch": nch, "calls": calls, "chunk_of": chunk_of, "totch": totch,
              "sidx_cols": sum(cl[3] * 128 // 16 for cl in calls)}
    return struct, in_maps


def _prep_weights(cfg, W_emb, b_emb, Wf, bf, Ws, bs, gamma, beta, W1, b1, W2, b2,
                  W_out, b_out, batch):
    """Replicated weight tensors, packed for the device layouts."""
    L, H, EF, G = cfg.L, cfg.H, cfg.EF, cfg.G
    f32 = np.float32
    # The f-half (gate) is NEGATED everywhere so that one Exp(z) yields
    # exp(-mf) on the f-half and exp(ms) on the s-half.
    wsrc = np.stack([np.concatenate([-Wf[l][H : 2 * H], Ws[l][H : 2 * H]], 1)
                     for l in range(L)]).astype(f32)          # [L,128,256]
    wdst = np.stack([np.concatenate([-Wf[l][:H], Ws[l][:H]], 1)
                     for l in range(L)]).astype(f32)          # [L,128,256]
    wc = np.zeros((L, 8, 2 * H), f32)
    wc[:, :EF, :H] = -np.asarray(Wf, f32)[:, 2 * H :, :]
    wc[:, :EF, H:] = np.asarray(Ws, f32)[:, 2 * H :, :]       # [L,8,256]
    bias_b = np.zeros((L, 128, 2 * H), f32)
    bias_b[:, :W, :H] = -np.asarray(bf, f32)[:, None, :]
    bias_b[:, :W, H:] = np.asarray(bs, f32)[:, None, :]       # [L,128,256]
    gb = np.zeros((L, 128, 2), f32)
    gb[:, :H, 0] = np.asarray(gamma, f32)
    gb[:, :H, 1] = np.asarray(beta, f32)
    cnt = np.bincount(np.asarray(batch, np.int64), minlength=G).astype(f32)
    invc = np.zeros((128, cfg.GH), f32)
    ic = 1.0 / np.maximum(cnt, 1.0)
    icp = np.zeros(cfg.GP, f32)
    icp[:G] = ic
    invc[:, :] = icp.reshape(cfg.GH, 128).T
    return {
        "wemb": np.asarray(W_emb, f32),                        # [NF,128]
        "bemb": np.asarray(b_emb, f32).reshape(cfg.H, 1),
        "wsrc": wsrc, "wdst": wdst, "wc": wc, "bias_b": bias_b, "gb": gb,
        "invc": invc,
        "w1": np.asarray(W1, f32), "b1": np.asarray(b1, f32).reshape(-1, 1),
        "w2": np.asarray(W2, f32), "b2": np.asarray(b2, f32).reshape(-1, 1),
        "wout": np.asarray(W_out, f32), "bout": np.asarray(b_out, f32).reshape(1, 1),
    }


def _build(cfg, struct):
    """Trace the bass program. Returns nc."""
    NPC, NW, NT, L, H, NF = cfg.NPC, cfg.NW, cfg.NT, cfg.L, cfg.H, cfg.NF
    GP, GH = cfg.GP, cfg.GH
    nch, calls, chunk_of, totch = (struct["nch"], struct["calls"],
                                   struct["chunk_of"], struct["totch"])
    AF = mybir.ActivationFunctionType

    nc = bacc.Bacc("TRN2", target_bir_lowering=False, debug=False,
                   num_devices=NCORES)

    # ---- kernel I/O ----
    d_ctab = nc.declare_dram_parameter("ctab", [totch, 128, 240], FP32, isOutput=False)
    d_sidx = nc.declare_dram_parameter("sidx", [128, struct["sidx_cols"]], I16, isOutput=False)
    d_xt = nc.declare_dram_parameter("xt", [NF, NPC], FP32, isOutput=False)
    d_ohg = nc.declare_dram_parameter("ohg", [NT, 128, GP], FP32, isOutput=False)
    d_tmask = nc.declare_dram_parameter("tmask", [128, 896], FP32, isOutput=False)
    d_wemb = nc.declare_dram_parameter("wemb", [NF, H], FP32, isOutput=False)
    d_bemb = nc.declare_dram_parameter("bemb", [H, 1], FP32, isOutput=False)
    d_wsrc = nc.declare_dram_parameter("wsrc", [L, H, 2 * H], FP32, isOutput=False)
    d_wdst = nc.declare_dram_parameter("wdst", [L, H, 2 * H], FP32, isOutput=False)
    d_wc = nc.declare_dram_parameter("wc", [L, 8, 2 * H], FP32, isOutput=False)
    d_biasb = nc.declare_dram_parameter("bias_b", [L, 128, 2 * H], FP32, isOutput=False)
    d_gb = nc.declare_dram_parameter("gb", [L, 128, 2], FP32, isOutput=False)
    d_invc = nc.declare_dram_parameter("invc", [128, GH], FP32, isOutput=False)
    d_w1 = nc.declare_dram_parameter("w1", [H, 64], FP32, isOutput=False)
    d_b1 = nc.declare_dram_parameter("b1", [64, 1], FP32, isOutput=False)
    d_w2 = nc.declare_dram_parameter("w2", [64, 32], FP32, isOutput=False)
    d_b2 = nc.declare_dram_parameter("b2", [32, 1], FP32, isOutput=False)
    d_wout = nc.declare_dram_parameter("wout", [32, 1], FP32, isOutput=False)
    d_bout = nc.declare_dram_parameter("bout", [1, 1], FP32, isOutput=False)
    d_out = nc.declare_dram_parameter("out", [GP], FP32, isOutput=True)

    # ---- internal DRAM (collectives) ----
    ag_in = [nc.dram_tensor(f"ag_in{l}", [NPC, 2 * H], FP32) for l in range(L)]
    ts_full = [nc.dram_tensor(f"ts_full{l}", [cfg.NP, 2 * H], FP32,
                              addr_space="Shared") for l in range(L)]
    ar_in = [nc.dram_tensor(f"ar_in{l}", [128, 2], FP32) for l in range(L)]
    ar_out = [nc.dram_tensor(f"ar_out{l}", [128, 2], FP32, addr_space="Shared")
              for l in range(L)]
    pool_in = nc.dram_tensor("pool_in", [GP, H], FP32)
    pool_out = nc.dram_tensor("pool_out", [GP, H], FP32, addr_space="Shared")
    rg = [list(range(NCORES))]

    with tile.TileContext(nc) as tc:
        with (
            tc.tile_pool(name="persist", bufs=1) as pp,
            tc.tile_pool(name="lw", bufs=3) as lwp,       # per-layer weights
            tc.tile_pool(name="chunk", bufs=6) as chp,    # comb/oht tiles
            tc.tile_pool(name="gath", bufs=6) as gap,     # gather outputs
            tc.tile_pool(name="sidxp", bufs=6) as sxp,    # gather index tiles
            tc.tile_pool(name="edge", bufs=6) as edp,
            tc.tile_pool(name="blk", bufs=2) as bkp,     # z/s1/s2/m tiles
            tc.tile_pool(name="tsout", bufs=3) as tso,    # T_src build staging
            tc.tile_pool(name="small", bufs=2) as smp,
            tc.tile_pool(name="psz", bufs=3, space="PSUM") as psz,
            tc.tile_pool(name="psagg", bufs=2, space="PSUM") as psagg,
            tc.tile_pool(name="psb", bufs=2, space="PSUM") as psb,
        ):
            h_T = pp.tile([128, NPC], FP32)
            xn_T = pp.tile([128, NPC], FP32)
            ident = pp.tile([128, 128], FP32)
            tmask_t = pp.tile([128, 896], FP32)

            nc.sync.dma_start(out=tmask_t[:], in_=d_tmask[:])
            make_identity(nc, ident[:])
            epsc = pp.tile([128, 1], FP32)
            nc.vector.memset(epsc[:], EPS)

            # ---------- embedding: h = relu(x @ W_emb + b_emb) ----------
            wemb_t = smp.tile([NF, H], FP32)
            bemb_t = pp.tile([H, 1], FP32)
            nc.sync.dma_start(out=wemb_t[:], in_=d_wemb[:])
            nc.sync.dma_start(out=bemb_t[:], in_=d_bemb[:])
            EMBW = 448
            for s in range(NPC // EMBW):
                xs_t = edp.tile([NF, EMBW], FP32, tag="xs")
                nc.sync.dma_start(out=xs_t[:],
                                  in_=d_xt[:, s * EMBW : (s + 1) * EMBW])
                ps = psb.tile([128, EMBW], FP32, space="PSUM", tag="ps")
                nc.tensor.matmul(out=ps[:], lhsT=wemb_t[:], rhs=xs_t[:],
                                 start=True, stop=True)
                nc.scalar.activation(out=h_T[:, s * EMBW : (s + 1) * EMBW],
                                     in_=ps[:], func=AF.Relu, bias=bemb_t[:, 0:1])

            # ---------- layers ----------
            for l in range(L):
                wsrc_t = lwp.tile([H, 2 * H], FP32, tag="wsrc")
                wdst_t = lwp.tile([H, 2 * H], FP32, tag="wdst")
                wc_t = lwp.tile([8, 2 * H], FP32, tag="wc")
                biasb_t = lwp.tile([128, 2 * H], FP32, tag="biasb")
                gb_t = lwp.tile([128, 2], FP32, tag="gb")
                nc.sync.dma_start(out=wsrc_t[:], in_=d_wsrc[l])
                nc.sync.dma_start(out=wdst_t[:], in_=d_wdst[l])
                nc.sync.dma_start(out=wc_t[:], in_=d_wc[l])
                nc.sync.dma_start(out=biasb_t[:], in_=d_biasb[l])
                nc.sync.dma_start(out=gb_t[:], in_=d_gb[l])

                # ---- T_src shard build + AllGather ----
                for t in range(NT):
                    ps = psb.tile([128, 2 * H], FP32, space="PSUM", tag="ps")
                    nc.tensor.matmul(out=ps[:], lhsT=h_T[:, t * 128 : (t + 1) * 128],
                                     rhs=wsrc_t[:], start=True, stop=True)
                    st = tso.tile([128, 2 * H], FP32)
                    nc.vector.tensor_copy(out=st[:], in_=ps[:])
                    nc.gpsimd.dma_start(out=ag_in[l][t * 128 : (t + 1) * 128, :],
                                        in_=st[:])
                nc.gpsimd.collective_compute(
                    "AllGather", mybir.AluOpType.bypass, replica_groups=rg,
                    ins=[ag_in[l][:]], outs=[ts_full[l][:]])

                # ---- per-window: Td build, gathers, edge chunks ----
                calls_by_w = {}
                for cl in calls:
                    calls_by_w.setdefault(cl[0], []).append(cl)
                chunks_by_w = {}
                for gi, (w, hf, k) in enumerate(chunk_of):
                    chunks_by_w.setdefault(w, []).append((gi, hf, k))
                scol = 0
                for w in range(NW):
                    wchunks = chunks_by_w.get(w, [])
                    if not wchunks:
                        nc.vector.tensor_copy(out=xn_T[:, w * W : (w + 1) * W],
                                              in_=h_T[:, w * W : (w + 1) * W])
                        continue
                    # rhs_cat for this window: [Td+bias(112); Wc(8); junk(8)]
                    ps = psb.tile([128, 2 * H], FP32, space="PSUM", tag="ps")
                    nc.tensor.matmul(out=ps[:W, :], lhsT=h_T[:, w * W : (w + 1) * W],
                                     rhs=wdst_t[:], start=True, stop=True)
                    rc = lwp.tile([128, 2 * H], FP32, tag="rc")
                    nc.vector.tensor_add(out=rc[0:W, :], in0=ps[:W, :],
                                         in1=biasb_t[0:W, :])
                    nc.sync.dma_start(out=rc[W : W + 8, :], in_=d_wc[l])
                    # src gathers for this window
                    gtiles = {}
                    for (_w, hf, k0, k) in calls_by_w.get(w, []):
                        nidx = k * 128
                        sx = sxp.tile([128, CALLCAP * 128 // 16], I16, tag="sx")
                        nc.sync.dma_start(out=sx[:, : nidx // 16],
                                          in_=d_sidx[:, scol : scol + nidx // 16])
                        scol += nidx // 16
                        g = gap.tile([128, CALLCAP, 2 * H], FP32, tag="gt")
                        base = ts_full[l][0 : min(SPLIT, cfg.NP), :] if hf == 0 \
                            else ts_full[l][SPLIT : cfg.NP, :]
                        nc.gpsimd.dma_gather(
                            out_ap=g[:, :k, :], in_ap=base,
                            idxs_ap=sx[:, : nidx // 16],
                            num_idxs=nidx, num_idxs_reg=nidx, elem_size=2 * H,
                            single_packet=False)
                        for kk in range(k):
                            gtiles[(hf, k0 + kk)] = (g, kk)
                    # edge chunks, processed in pairs (even count per window)
                    agg_ps = psagg.tile([128, W], FP32, space="PSUM", tag="agg")
                    npair = len(wchunks) // 2
                    for pi in range(npair):
                        gi, hf, k = wchunks[2 * pi]
                        ct = chp.tile([128, 2, 240], FP32, tag="ct")
                        nc.sync.dma_start(
                            out=ct[:],
                            in_=d_ctab[gi : gi + 2].rearrange("c p e -> p c e"))
                        pz = psz.tile([128, 2, 2 * H], FP32, space="PSUM", tag="pz")
                        for j in range(2):
                            nc.tensor.matmul(out=pz[:, j, :],
                                             lhsT=ct[:, j, 0:128], rhs=rc[:],
                                             start=True, stop=True)
                        g, kk = gtiles[(hf, k)]
                        z = edp.tile([128, 2, 2 * H], FP32, tag="z")
                        nc.vector.tensor_add(out=z[:], in0=pz[:],
                                             in1=g[:, kk : kk + 2, :])
                        # z = [-mf | ms] per chunk; e = exp(z); then
                        # m = ln(1+e_s) / (1+e_f)  (= softplus(ms)*sigmoid(mf))
                        e = edp.tile([128, 2, 2 * H], FP32, tag="e")
                        nc.scalar.activation(out=e[:], in_=z[:], func=AF.Exp)
                        t = edp.tile([128, 2, H], FP32, tag="t")
                        nc.scalar.activation(out=t[:], in_=e[:, :, H:], func=AF.Ln,
                                             bias=1.0)
                        d = edp.tile([128, 2, H], FP32, tag="d")
                        nc.vector.tensor_scalar_add(out=d[:], in0=e[:, :, :H],
                                                    scalar1=1.0)
                        r = edp.tile([128, 2, H], FP32, tag="r")
                        nc.vector.reciprocal_approx_fast(out=r[:], in_=d[:])
                        m = edp.tile([128, 2, H], FP32, tag="m")
                        nc.vector.tensor_mul(out=m[:], in0=t[:], in1=r[:])
                        for j in range(2):
                            nc.tensor.matmul(out=agg_ps[:], lhsT=m[:, j, :],
                                             rhs=ct[:, j, 128:240],
                                             start=(pi == 0 and j == 0),
                                             stop=(pi == npair - 1 and j == 1),
                                             skip_group_check=True)
                    nc.vector.tensor_add(out=xn_T[:, w * W : (w + 1) * W],
                                         in0=agg_ps[:],
                                         in1=h_T[:, w * W : (w + 1) * W])

                # ---- BatchNorm stats + AllReduce ----
                NB = NPC // 896
                part = smp.tile([128, 32], FP32, tag="part")
                for b in range(NB):
                    blk = xn_T[:, b * 896 : (b + 1) * 896]
                    nc.vector.reduce_sum(out=part[:, b : b + 1], in_=blk,
                                         axis=mybir.AxisListType.X)
                    sqb = bkp.tile([128, 896], FP32, tag="sqb")
                    nc.vector.tensor_mul(out=sqb[:], in0=blk, in1=blk)
                    nc.vector.reduce_sum(out=part[:, 16 + b : 17 + b], in_=sqb[:],
                                         axis=mybir.AxisListType.X)
                stats = smp.tile([128, 2], FP32, tag="stats")
                nc.vector.reduce_sum(out=stats[:, 0:1], in_=part[:, 0:NB],
                                     axis=mybir.AxisListType.X)
                nc.vector.reduce_sum(out=stats[:, 1:2], in_=part[:, 16 : 16 + NB],
                                     axis=mybir.AxisListType.X)
                nc.gpsimd.dma_start(out=ar_in[l][:], in_=stats[:])
                nc.gpsimd.collective_compute(
                    "AllReduce", mybir.AluOpType.add, replica_groups=rg,
                    ins=[ar_in[l][:]], outs=[ar_out[l][:]])
                st2 = smp.tile([128, 2], FP32, tag="st2")
                nc.gpsimd.dma_start(out=st2[:], in_=ar_out[l][:])
                mu = smp.tile([128, 1], FP32, tag="mu")
                var = smp.tile([128, 1], FP32, tag="var")
                inv = smp.tile([128, 1], FP32, tag="inv")
                sc = smp.tile([128, 1], FP32, tag="sc")
                bi = smp.tile([128, 1], FP32, tag="bi")
                rN = 1.0 / cfg.N
                nc.vector.tensor_scalar_mul(out=mu[:], in0=st2[:, 0:1], scalar1=rN)
                nc.vector.tensor_scalar_mul(out=var[:], in0=st2[:, 1:2], scalar1=rN)
                nc.vector.tensor_mul(out=inv[:], in0=mu[:], in1=mu[:])
                nc.vector.tensor_sub(out=var[:], in0=var[:], in1=inv[:])
                # 1/sqrt(var+eps) = exp(-0.5*ln(var+eps)) -- stays on Exp/Ln table
                nc.scalar.activation(out=inv[:], in_=var[:], func=AF.Ln, bias=epsc[:, 0:1])
                nc.scalar.activation(out=inv[:], in_=inv[:], func=AF.Exp, scale=-0.5)
                nc.vector.tensor_mul(out=sc[:], in0=gb_t[:, 0:1], in1=inv[:])
                nc.vector.tensor_mul(out=bi[:], in0=mu[:], in1=sc[:])
                nc.vector.tensor_sub(out=bi[:], in0=gb_t[:, 1:2], in1=bi[:])
                # relu pass + masked residual update (blockwise)
                for b in range(NB):
                    blk = slice(b * 896, (b + 1) * 896)
                    rt = bkp.tile([128, 896], FP32, tag="sqb")
                    nc.scalar.activation(out=rt[:], in_=xn_T[:, blk], func=AF.Relu,
                                         scale=sc[:, 0:1], bias=bi[:, 0:1])
                    if b == NB - 1:
                        nc.vector.tensor_mul(out=rt[:], in0=rt[:], in1=tmask_t[:])
                    nc.vector.tensor_add(out=h_T[:, blk], in0=h_T[:, blk],
                                         in1=rt[:])

            # ---------- pooling ----------
            pool_ps = []
            for _gh in range(GH):
                pacc = psb.tile([128, H], FP32, space="PSUM", tag="ps", name=f"pacc{_gh}")
                pool_ps.append(pacc)
            for t in range(NT):
                ohg_t = chp.tile([128, GP], FP32, tag="ohg")
                nc.sync.dma_start(out=ohg_t[:], in_=d_ohg[t])
                tps = psz.tile([128, 128], FP32, space="PSUM", tag="pz")
                nc.tensor.transpose(out=tps[:], in_=h_T[:, t * 128 : (t + 1) * 128],
                                    identity=ident[:])
                hn = edp.tile([128, 128], FP32, tag="hn")
                nc.vector.tensor_copy(out=hn[:], in_=tps[:])
                for gh in range(GH):
                    nc.tensor.matmul(out=pool_ps[gh][:],
                                     lhsT=ohg_t[:, gh * 128 : (gh + 1) * 128],
                                     rhs=hn[:], start=(t == 0), stop=(t == NT - 1),
                                     skip_group_check=True)
            for gh in range(GH):
                pt = tso.tile([128, H], FP32, tag="pt")
                nc.vector.tensor_copy(out=pt[:], in_=pool_ps[gh][:])
                nc.gpsimd.dma_start(out=pool_in[gh * 128 : (gh + 1) * 128, :],
                                    in_=pt[:])
            nc.gpsimd.collective_compute(
                "AllReduce", mybir.AluOpType.add, replica_groups=rg,
                ins=[pool_in[:]], outs=[pool_out[:]])

            # ---------- readout MLP (replicated) ----------
            invc_t = smp.tile([128, GH], FP32, tag="invc")
            nc.sync.dma_start(out=invc_t[:], in_=d_invc[:])
            pooled_T = pp.tile([128, GP], FP32)
            for gh in range(GH):
                q = edp.tile([128, H], FP32, tag="q")
                nc.gpsimd.dma_start(out=q[:], in_=pool_out[gh * 128 : (gh + 1) * 128, :])
                nc.vector.tensor_scalar_mul(out=q[:], in0=q[:],
                                            scalar1=invc_t[:, gh : gh + 1])
                tps = psz.tile([128, 128], FP32, space="PSUM", tag="pz")
                nc.tensor.transpose(out=tps[:], in_=q[:], identity=ident[:])
                nc.vector.tensor_copy(out=pooled_T[:, gh * 128 : (gh + 1) * 128],
                                      in_=tps[:])
            w1_t = smp.tile([H, 64], FP32, tag="w1")
            w2_t = smp.tile([64, 32], FP32, tag="w2")
            wout_t = smp.tile([32, 1], FP32, tag="wout")
            b1_t = smp.tile([64, 1], FP32, tag="b1")
            b2_t = smp.tile([32, 1], FP32, tag="b2")
            bout_t = smp.tile([1, 1], FP32, tag="bout")
            for tt, dd in ((w1_t, d_w1), (w2_t, d_w2), (wout_t, d_wout),
                           (b1_t, d_b1), (b2_t, d_b2), (bout_t, d_bout)):
                nc.sync.dma_start(out=tt[:], in_=dd[:])
            g1ps = psb.tile([64, GP], FP32, space="PSUM", tag="ps")
            nc.tensor.matmul(out=g1ps[:], lhsT=w1_t[:], rhs=pooled_T[:],
                             start=True, stop=True)
            g1 = edp.tile([64, GP], FP32, tag="g1")
            nc.scalar.activation(out=g1[:], in_=g1ps[:], func=AF.Relu,
                                 bias=b1_t[:, 0:1])
            g2ps = psb.tile([32, GP], FP32, space="PSUM", tag="ps")
            nc.tensor.matmul(out=g2ps[:], lhsT=w2_t[:], rhs=g1[:],
                             start=True, stop=True)
            g2 = edp.tile([32, GP], FP32, tag="g2")
            nc.scalar.activation(out=g2[:], in_=g2ps[:], func=AF.Relu,
                                 bias=b2_t[:, 0:1])
            ops = psb.tile([1, GP], FP32, space="PSUM", tag="ps")
            nc.tensor.matmul(out=ops[:], lhsT=wout_t[:], rhs=g2[:],
                             start=True, stop=True)
            ot = edp.tile([1, GP], FP32, tag="ot")
            nc.scalar.activation(out=ot[:], in_=ops[:], func=AF.Identity,
                                 bias=bout_t[:, 0:1])
            nc.sync.dma_start(out=d_out[None, :], in_=ot[:])

    # Pin all activations to the one table containing {exp, ln, relu,
    # identity, copy} so no ACT_TABLE_LOAD thrash occurs between Exp and Ln.
    import concourse.bacc as _bacc_mod
    _orig_gat = _bacc_mod.get_activation_tables

    def _pinned(arch):
        tabs = _orig_gat(arch)
        keep = "natural_log_exp_and_others"
        assert keep in tabs
        return {k: (v if k == keep else set()) for k, v in tabs.items()}

    _bacc_mod.get_activation_tables = _pinned
    try:
        nc.finalize()
    finally:
        _bacc_mod.get_activation_tables = _orig_gat
    return nc


def _run(cfg, inputs, trace=False):
    struct, in_maps = _prep(cfg, inputs["x"], inputs["edge_index"],
                            inputs["edge_attr"], inputs["batch"])
    wmap = _prep_weights(cfg, inputs["W_emb"], inputs["b_emb"], inputs["Wf"],
                         inputs["bf"], inputs["Ws"], inputs["bs"],
                         inputs["gamma"], inputs["beta"], inputs["W1"],
                         inputs["b1"], inputs["W2"], inputs["b2"],
                         inputs["W_out"], inputs["b_out"], inputs["batch"])
    for m in in_maps:
        m.update(wmap)
    nc = _build(cfg, struct)
    res = run_bass_kernel_spmd(nc, in_maps, list(range(NCORES)), trace=trace)
    out = res.results[0]["out"][: cfg.G].astype(np.float32)
    return out, res


def kernel(**inputs):
    x = np.asarray(inputs["x"])
    ei = np.asarray(inputs["edge_index"])
    batch = np.asarray(inputs["batch"])
    cfg = Cfg(N=x.shape[0], E=ei.shape[1], G=256)
    out, _ = _run(cfg, inputs)
    return out.astype(np.float32)


# revision 18
# speedup vs baseline: 2.1389x; 1.1211x over previous
"""CGCNN message-passing kernel for 8 Trainium2 NeuronCores.

Strategy (per core, per layer):
  - Nodes are sharded contiguously across cores (padded so the shard size is a
    multiple of lcm(112,128)=896). h lives feature-major ([128 feat, NPC nodes])
    in SBUF for the whole kernel.
  - Edges are sharded by dst shard, grouped into 112-node dst windows, and
    padded to 128-edge chunks on the host. All index-derived structures
    (onehots, gather indices, edge attrs) are precomputed on the host into
    dense input tensors; the device kernel is pure dense compute.
  - dst-side projections + edge-attr term: PE matmul with a host-built
    combined stationary [onehot(112); ea^T(8); 0(8)] against
    [Td_window; Wc; 0] — no gather needed (dst is shard-local).
  - src-side projections: one AllGather of the per-shard projection table
    T_src = h @ [Wf_b|Ws_b], then dma_gather (SWDGE) of 1KB rows per edge.
  - segment-sum: PE matmul m^T @ onehot accumulating in PSUM per window.
  - BatchNorm: feature-major reductions + a tiny AllReduce of [sum, sumsq].
  - Pooling: PE onehot matmul per node tile + AllReduce, then the small MLP
    replicated on every core.
"""

import math
import numpy as np

import concourse.bacc as bacc
import concourse.bass as bass
import concourse.tile as tile
import concourse.mybir as mybir
from concourse.bass_utils import run_bass_kernel_spmd
from concourse.masks import make_identity

FP32 = mybir.dt.float32
BF16 = mybir.dt.bfloat16
I16 = mybir.dt.int16

NCORES = 8
W = 112          # dst window (nodes); onehot rows 0..111, ea rows 112..119
SPLIT = 32768    # int16 dma_gather index limit -> lo/hi table split
CALLCAP = 4      # max 128-edge chunks per dma_gather call
EPS = 1e-5


class Cfg:
    def __init__(self, N, E, G, NF=16, EF=8, H=128, L=4):
        self.N, self.E, self.G, self.NF, self.EF, self.H, self.L = N, E, G, NF, EF, H, L
        per = math.ceil(N / NCORES)
        self.NPC = math.ceil(per / 896) * 896      # shard size (mult of 112 & 128)
        self.NP = self.NPC * NCORES                # padded node count
        self.NW = self.NPC // W                    # windows per shard
        self.NT = self.NPC // 128                  # 128-node tiles per shard
        self.GP = math.ceil(G / 128) * 128         # padded graph count
        self.GH = self.GP // 128                   # graph halves


def _prep(cfg, x, edge_index, edge_attr, batch):
    """Host-side preprocessing. Returns (structure, per-core input maps)."""
    N, E, G = cfg.N, cfg.E, cfg.G
    NPC, NW = cfg.NPC, cfg.NW
    src = np.asarray(edge_index[0], dtype=np.int64)
    dst = np.asarray(edge_index[1], dtype=np.int64)
    ea = np.asarray(edge_attr, dtype=np.float32)

    core = dst // NPC
    win = (dst % NPC) // W
    half = (src >= SPLIT).astype(np.int64)
    # group id per edge; sort once
    gid = (core * NW + win) * 2 + half
    order = np.argsort(gid * E + np.arange(E), kind="stable")  # stable group sort
    gid_s, src_s, dst_s, ea_s = gid[order], src[order], dst[order], ea[order]

    ngroups = NCORES * NW * 2
    cnt = np.bincount(gid_s, minlength=ngroups).reshape(NCORES, NW, 2)
    # global (uniform across cores) chunk counts per (win, half)
    nch = np.ceil(cnt.max(axis=0) / 128).astype(np.int64)  # [NW, 2]
    nch = ((nch + 1) // 2) * 2  # even so edge chunks can be processed in pairs
    # call split per (win, half)
    calls = []  # list of (w, half, chunk0, nchunks) in emission order
    chunk_of = []  # (w, half, k) for each global chunk index, in order
    for w in range(NW):
        for hf in range(2):
            n = int(nch[w, hf])
            k0 = 0
            while k0 < n:
                k = min(CALLCAP, n - k0)
                calls.append((w, hf, k0, k))
                k0 += k
            for k in range(n):
                chunk_of.append((w, hf, k))
    totch = len(chunk_of)

    group_start = np.zeros(ngroups + 1, dtype=np.int64)
    np.cumsum(np.bincount(gid_s, minlength=ngroups), out=group_start[1:])

    in_maps = []
    EFp = cfg.EF
    for c in range(NCORES):
        # ctab: cols 0..127 = comb ([onehot(112); ea(EF); 0]), cols 128..239 = oht
        ctab = np.zeros((totch, 128, 240), np.float32)
        sidx_cols = []
        for gi, (w, hf, k) in enumerate(chunk_of):
            g = (c * NW + w) * 2 + hf
            s0, s1 = group_start[g], group_start[g + 1]
            e0 = s0 + k * 128
            e1 = min(s1, e0 + 128)
            if e1 > e0:
                n_e = e1 - e0
                dl = (dst_s[e0:e1] - (c * NPC + w * W)).astype(np.int64)
                ee = np.arange(n_e)
                ctab[gi, dl, ee] = 1.0
                ctab[gi, W : W + EFp, :n_e] = ea_s[e0:e1].T
                ctab[gi, ee, 128 + dl] = 1.0
        # gather indices per call
        for (w, hf, k0, k) in calls:
            g = (c * NW + w) * 2 + hf
            s0, s1 = group_start[g], group_start[g + 1]
            nidx = k * 128
            iv = np.zeros(nidx, np.int64)
            e0 = s0 + k0 * 128
            e1 = min(s1, e0 + nidx)
            if e1 > e0:
                iv[: e1 - e0] = src_s[e0:e1] - hf * SPLIT
            sidx_cols.append(iv.reshape(nidx // 16, 16).T.astype(np.int16))
        sidx = np.tile(np.concatenate(sidx_cols, axis=1), (8, 1))

        # node features, transposed + sharded
        xt = np.zeros((cfg.NF, NPC), np.float32)
        lo, hi = c * NPC, min((c + 1) * NPC, N)
        if hi > lo:
            xt[:, : hi - lo] = np.asarray(x[lo:hi], np.float32).T

        # pooling onehot [NT, 128, GP] and tail mask [128, 896]
        ohg = np.zeros((cfg.NT, 128, cfg.GP), np.float32)
        if hi > lo:
            nn = np.arange(lo, hi)
            b = np.asarray(batch[lo:hi], dtype=np.int64)
            ohg[(nn - lo) // 128, (nn - lo) % 128, b] = 1.0
        tmask = np.zeros((128, 896), np.float32)
        nreal = max(0, min(NPC, N - c * NPC))
        k = max(0, nreal - (NPC - 896))
        tmask[:, :k] = 1.0

        in_maps.append(
            {"ctab": ctab, "sidx": sidx, "xt": xt, "ohg": ohg, "tmask": tmask}
        )

    struct = {"nch": nch, "calls": calls, "chunk_of": chunk_of, "totch": totch,
              "sidx_cols": sum(cl[3] * 128 // 16 for cl in calls)}
    return struct, in_maps


def _prep_weights(cfg, W_emb, b_emb, Wf, bf, Ws, bs, gamma, beta, W1, b1, W2, b2,
                  W_out, b_out, batch):
    """Replicated weight tensors, packed for the device layouts."""
    L, H, EF, G = cfg.L, cfg.H, cfg.EF, cfg.G
    f32 = np.float32
    # The f-half (gate) is NEGATED everywhere so that one Exp(z) yields
    # exp(-mf) on the f-half and exp(ms) on the s-half.
    wsrc = np.stack([np.concatenate([-Wf[l][H : 2 * H], Ws[l][H : 2 * H]], 1)
                     for l in range(L)]).astype(f32)          # [L,128,256]
    wdst = np.stack([np.concatenate([-Wf[l][:H], Ws[l][:H]], 1)
                     for l in range(L)]).astype(f32)          # [L,128,256]
    wc = np.zeros((L, 8, 2 * H), f32)
    wc[:, :EF, :H] = -np.asarray(Wf, f32)[:, 2 * H :, :]
    wc[:, :EF, H:] = np.asarray(Ws, f32)[:, 2 * H :, :]       # [L,8,256]
    bias_b = np.zeros((L, 128, 2 * H), f32)
    bias_b[:, :W, :H] = -np.asarray(bf, f32)[:, None, :]
    bias_b[:, :W, H:] = np.asarray(bs, f32)[:, None, :]       # [L,128,256]
    gb = np.zeros((L, 128, 2), f32)
    gb[:, :H, 0] = np.asarray(gamma, f32)
    gb[:, :H, 1] = np.asarray(beta, f32)
    cnt = np.bincount(np.asarray(batch, np.int64), minlength=G).astype(f32)
    invc = np.zeros((128, cfg.GH), f32)
    ic = 1.0 / np.maximum(cnt, 1.0)
    icp = np.zeros(cfg.GP, f32)
    icp[:G] = ic
    invc[:, :] = icp.reshape(cfg.GH, 128).T
    return {
        "wemb": np.asarray(W_emb, f32),                        # [NF,128]
        "bemb": np.asarray(b_emb, f32).reshape(cfg.H, 1),
        "wsrc": wsrc, "wdst": wdst, "wc": wc, "bias_b": bias_b, "gb": gb,
        "invc": invc,
        "w1": np.asarray(W1, f32), "b1": np.asarray(b1, f32).reshape(-1, 1),
        "w2": np.asarray(W2, f32), "b2": np.asarray(b2, f32).reshape(-1, 1),
        "wout": np.asarray(W_out, f32), "bout": np.asarray(b_out, f32).reshape(1, 1),
    }


def _build(cfg, struct):
    """Trace the bass program. Returns nc."""
    NPC, NW, NT, L, H, NF = cfg.NPC, cfg.NW, cfg.NT, cfg.L, cfg.H, cfg.NF
    GP, GH = cfg.GP, cfg.GH
    nch, calls, chunk_of, totch = (struct["nch"], struct["calls"],
                                   struct["chunk_of"], struct["totch"])
    AF = mybir.ActivationFunctionType

    nc = bacc.Bacc("TRN2", target_bir_lowering=False, debug=False,
                   num_devices=NCORES)

    # ---- kernel I/O ----
    d_ctab = nc.declare_dram_parameter("ctab", [totch, 128, 240], FP32, isOutput=False)
    d_sidx = nc.declare_dram_parameter("sidx", [128, struct["sidx_cols"]], I16, isOutput=False)
    d_xt = nc.declare_dram_parameter("xt", [NF, NPC], FP32, isOutput=False)
    d_ohg = nc.declare_dram_parameter("ohg", [NT, 128, GP], FP32, isOutput=False)
    d_tmask = nc.declare_dram_parameter("tmask", [128, 896], FP32, isOutput=False)
    d_wemb = nc.declare_dram_parameter("wemb", [NF, H], FP32, isOutput=False)
    d_bemb = nc.declare_dram_parameter("bemb", [H, 1], FP32, isOutput=False)
    d_wsrc = nc.declare_dram_parameter("wsrc", [L, H, 2 * H], FP32, isOutput=False)
    d_wdst = nc.declare_dram_parameter("wdst", [L, H, 2 * H], FP32, isOutput=False)
    d_wc = nc.declare_dram_parameter("wc", [L, 8, 2 * H], FP32, isOutput=False)
    d_biasb = nc.declare_dram_parameter("bias_b", [L, 128, 2 * H], FP32, isOutput=False)
    d_gb = nc.declare_dram_parameter("gb", [L, 128, 2], FP32, isOutput=False)
    d_invc = nc.declare_dram_parameter("invc", [128, GH], FP32, isOutput=False)
    d_w1 = nc.declare_dram_parameter("w1", [H, 64], FP32, isOutput=False)
    d_b1 = nc.declare_dram_parameter("b1", [64, 1], FP32, isOutput=False)
    d_w2 = nc.declare_dram_parameter("w2", [64, 32], FP32, isOutput=False)
    d_b2 = nc.declare_dram_parameter("b2", [32, 1], FP32, isOutput=False)
    d_wout = nc.declare_dram_parameter("wout", [32, 1], FP32, isOutput=False)
    d_bout = nc.declare_dram_parameter("bout", [1, 1], FP32, isOutput=False)
    d_out = nc.declare_dram_parameter("out", [GP], FP32, isOutput=True)

    # ---- internal DRAM (collectives) ----
    ag_in = [nc.dram_tensor(f"ag_in{l}", [NPC, 2 * H], BF16) for l in range(L)]
    ts_full = [nc.dram_tensor(f"ts_full{l}", [cfg.NP, 2 * H], BF16,
                              addr_space="Shared") for l in range(L)]
    ar_in = [nc.dram_tensor(f"ar_in{l}", [128, 2], FP32) for l in range(L)]
    ar_out = [nc.dram_tensor(f"ar_out{l}", [128, 2], FP32, addr_space="Shared")
              for l in range(L)]
    pool_in = nc.dram_tensor("pool_in", [GP, H], FP32)
    pool_out = nc.dram_tensor("pool_out", [GP, H], FP32, addr_space="Shared")
    rg = [list(range(NCORES))]

    with tile.TileContext(nc) as tc:
        with (
            tc.tile_pool(name="persist", bufs=1) as pp,
            tc.tile_pool(name="lw", bufs=3) as lwp,       # per-layer weights
            tc.tile_pool(name="chunk", bufs=6) as chp,    # comb/oht tiles
            tc.tile_pool(name="gath", bufs=10) as gap,     # gather outputs
            tc.tile_pool(name="sidxp", bufs=6) as sxp,    # gather index tiles
            tc.tile_pool(name="edge", bufs=6) as edp,
            tc.tile_pool(name="blk", bufs=2) as bkp,     # z/s1/s2/m tiles
            tc.tile_pool(name="tsout", bufs=3) as tso,    # T_src build staging
            tc.tile_pool(name="small", bufs=2) as smp,
            tc.tile_pool(name="psz", bufs=3, space="PSUM") as psz,
            tc.tile_pool(name="psagg", bufs=2, space="PSUM") as psagg,
            tc.tile_pool(name="psb", bufs=2, space="PSUM") as psb,
        ):
            h_T = pp.tile([128, NPC], FP32)
            xn_T = pp.tile([128, NPC], FP32)
            ident = pp.tile([128, 128], FP32)
            tmask_t = pp.tile([128, 896], FP32)

            nc.sync.dma_start(out=tmask_t[:], in_=d_tmask[:])
            make_identity(nc, ident[:])
            epsc = pp.tile([128, 1], FP32)
            nc.vector.memset(epsc[:], EPS)

            # ---------- embedding: h = relu(x @ W_emb + b_emb) ----------
            wemb_t = smp.tile([NF, H], FP32)
            bemb_t = pp.tile([H, 1], FP32)
            nc.sync.dma_start(out=wemb_t[:], in_=d_wemb[:])
            nc.sync.dma_start(out=bemb_t[:], in_=d_bemb[:])
            EMBW = 448
            for s in range(NPC // EMBW):
                xs_t = edp.tile([NF, EMBW], FP32, tag="xs")
                nc.sync.dma_start(out=xs_t[:],
                                  in_=d_xt[:, s * EMBW : (s + 1) * EMBW])
                ps = psb.tile([128, EMBW], FP32, space="PSUM", tag="ps")
                nc.tensor.matmul(out=ps[:], lhsT=wemb_t[:], rhs=xs_t[:],
                                 start=True, stop=True)
                nc.scalar.activation(out=h_T[:, s * EMBW : (s + 1) * EMBW],
                                     in_=ps[:], func=AF.Relu, bias=bemb_t[:, 0:1])

            # ---------- layers ----------
            for l in range(L):
                wsrc_t = lwp.tile([H, 2 * H], FP32, tag="wsrc")
                wdst_t = lwp.tile([H, 2 * H], FP32, tag="wdst")
                wc_t = lwp.tile([8, 2 * H], FP32, tag="wc")
                biasb_t = lwp.tile([128, 2 * H], FP32, tag="biasb")
                gb_t = lwp.tile([128, 2], FP32, tag="gb")
                nc.sync.dma_start(out=wsrc_t[:], in_=d_wsrc[l])
                nc.sync.dma_start(out=wdst_t[:], in_=d_wdst[l])
                nc.sync.dma_start(out=wc_t[:], in_=d_wc[l])
                nc.sync.dma_start(out=biasb_t[:], in_=d_biasb[l])
                nc.sync.dma_start(out=gb_t[:], in_=d_gb[l])

                # ---- T_src shard build + AllGather ----
                for t in range(NT):
                    ps = psb.tile([128, 2 * H], FP32, space="PSUM", tag="ps")
                    nc.tensor.matmul(out=ps[:], lhsT=h_T[:, t * 128 : (t + 1) * 128],
                                     rhs=wsrc_t[:], start=True, stop=True)
                    st = tso.tile([128, 2 * H], BF16)
                    nc.vector.tensor_copy(out=st[:], in_=ps[:])
                    nc.gpsimd.dma_start(out=ag_in[l][t * 128 : (t + 1) * 128, :],
                                        in_=st[:])
                nc.gpsimd.collective_compute(
                    "AllGather", mybir.AluOpType.bypass, replica_groups=rg,
                    ins=[ag_in[l][:]], outs=[ts_full[l][:]])

                # ---- per-window: Td build, gathers, edge chunks ----
                calls_by_w = {}
                for cl in calls:
                    calls_by_w.setdefault(cl[0], []).append(cl)
                chunks_by_w = {}
                for gi, (w, hf, k) in enumerate(chunk_of):
                    chunks_by_w.setdefault(w, []).append((gi, hf, k))
                scol = 0
                for w in range(NW):
                    wchunks = chunks_by_w.get(w, [])
                    if not wchunks:
                        nc.vector.tensor_copy(out=xn_T[:, w * W : (w + 1) * W],
                                              in_=h_T[:, w * W : (w + 1) * W])
                        continue
                    # rhs_cat for this window: [Td+bias(112); Wc(8); junk(8)]
                    ps = psb.tile([128, 2 * H], FP32, space="PSUM", tag="ps")
                    nc.tensor.matmul(out=ps[:W, :], lhsT=h_T[:, w * W : (w + 1) * W],
                                     rhs=wdst_t[:], start=True, stop=True)
                    rc = lwp.tile([128, 2 * H], FP32, tag="rc")
                    nc.vector.tensor_add(out=rc[0:W, :], in0=ps[:W, :],
                                         in1=biasb_t[0:W, :])
                    nc.sync.dma_start(out=rc[W : W + 8, :], in_=d_wc[l])
                    # src gathers for this window
                    gtiles = {}
                    for (_w, hf, k0, k) in calls_by_w.get(w, []):
                        nidx = k * 128
                        sx = sxp.tile([128, CALLCAP * 128 // 16], I16, tag="sx")
                        nc.sync.dma_start(out=sx[:, : nidx // 16],
                                          in_=d_sidx[:, scol : scol + nidx // 16])
                        scol += nidx // 16
                        g = gap.tile([128, CALLCAP, 2 * H], BF16, tag="gt")
                        base = ts_full[l][0 : min(SPLIT, cfg.NP), :] if hf == 0 \
                            else ts_full[l][SPLIT : cfg.NP, :]
                        nc.gpsimd.dma_gather(
                            out_ap=g[:, :k, :], in_ap=base,
                            idxs_ap=sx[:, : nidx // 16],
                            num_idxs=nidx, num_idxs_reg=nidx, elem_size=2 * H,
                            single_packet=False)
                        for kk in range(k):
                            gtiles[(hf, k0 + kk)] = (g, kk)
                    # edge chunks, processed in pairs (even count per window)
                    agg_ps = psagg.tile([128, W], FP32, space="PSUM", tag="agg")
                    npair = len(wchunks) // 2
                    for pi in range(npair):
                        gi, hf, k = wchunks[2 * pi]
                        ct = chp.tile([128, 2, 240], FP32, tag="ct")
                        nc.sync.dma_start(
                            out=ct[:],
                            in_=d_ctab[gi : gi + 2].rearrange("c p e -> p c e"))
                        pz = psz.tile([128, 2, 2 * H], FP32, space="PSUM", tag="pz")
                        for j in range(2):
                            nc.tensor.matmul(out=pz[:, j, :],
                                             lhsT=ct[:, j, 0:128], rhs=rc[:],
                                             start=True, stop=True)
                        g, kk = gtiles[(hf, k)]
                        z = edp.tile([128, 2, 2 * H], FP32, tag="z")
                        nc.vector.tensor_add(out=z[:], in0=pz[:],
                                             in1=g[:, kk : kk + 2, :])
                        # z = [-mf | ms] per chunk; e = exp(z); then
                        # m = ln(1+e_s) / (1+e_f)  (= softplus(ms)*sigmoid(mf))
                        e = edp.tile([128, 2, 2 * H], FP32, tag="e")
                        nc.scalar.activation(out=e[:], in_=z[:], func=AF.Exp)
                        t = edp.tile([128, 2, H], FP32, tag="t")
                        nc.scalar.activation(out=t[:], in_=e[:, :, H:], func=AF.Ln,
                                             bias=1.0)
                        d = edp.tile([128, 2, H], FP32, tag="d")
                        nc.vector.tensor_scalar_add(out=d[:], in0=e[:, :, :H],
                                                    scalar1=1.0)
                        r = edp.tile([128, 2, H], FP32, tag="r")
                        nc.vector.reciprocal_approx_fast(out=r[:], in_=d[:])
                        m = edp.tile([128, 2, H], FP32, tag="m")
                        nc.vector.tensor_mul(out=m[:], in0=t[:], in1=r[:])
                        for j in range(2):
                            nc.tensor.matmul(out=agg_ps[:], lhsT=m[:, j, :],
                                             rhs=ct[:, j, 128:240],
                                             start=(pi == 0 and j == 0),
                                             stop=(pi == npair - 1 and j == 1),
                                             skip_group_check=True)
                    nc.vector.tensor_add(out=xn_T[:, w * W : (w + 1) * W],
                                         in0=agg_ps[:],
                                         in1=h_T[:, w * W : (w + 1) * W])

                # ---- BatchNorm stats + AllReduce ----
                NB = NPC // 896
                part = smp.tile([128, 32], FP32, tag="part")
                for b in range(NB):
                    blk = xn_T[:, b * 896 : (b + 1) * 896]
                    nc.vector.reduce_sum(out=part[:, b : b + 1], in_=blk,
                                         axis=mybir.AxisListType.X)
                    sqb = bkp.tile([128, 896], FP32, tag="sqb")
                    nc.vector.tensor_mul(out=sqb[:], in0=blk, in1=blk)
                    nc.vector.reduce_sum(out=part[:, 16 + b : 17 + b], in_=sqb[:],
                                         axis=mybir.AxisListType.X)
                stats = smp.tile([128, 2], FP32, tag="stats")
                nc.vector.reduce_sum(out=stats[:, 0:1], in_=part[:, 0:NB],
                                     axis=mybir.AxisListType.X)
                nc.vector.reduce_sum(out=stats[:, 1:2], in_=part[:, 16 : 16 + NB],
                                     axis=mybir.AxisListType.X)
                nc.gpsimd.dma_start(out=ar_in[l][:], in_=stats[:])
                nc.gpsimd.collective_compute(
                    "AllReduce", mybir.AluOpType.add, replica_groups=rg,
                    ins=[ar_in[l][:]], outs=[ar_out[l][:]])
                st2 = smp.tile([128, 2], FP32, tag="st2")
                nc.gpsimd.dma_start(out=st2[:], in_=ar_out[l][:])
                mu = smp.tile([128, 1], FP32, tag="mu")
                var = smp.tile([128, 1], FP32, tag="var")
                inv = smp.tile([128, 1], FP32, tag="inv")
                sc = smp.tile([128, 1], FP32, tag="sc")
                bi = smp.tile([128, 1], FP32, tag="bi")
                rN = 1.0 / cfg.N
                nc.vector.tensor_scalar_mul(out=mu[:], in0=st2[:, 0:1], scalar1=rN)
                nc.vector.tensor_scalar_mul(out=var[:], in0=st2[:, 1:2], scalar1=rN)
                nc.vector.tensor_mul(out=inv[:], in0=mu[:], in1=mu[:])
                nc.vector.tensor_sub(out=var[:], in0=var[:], in1=inv[:])
                # 1/sqrt(var+eps) = exp(-0.5*ln(var+eps)) -- stays on Exp/Ln table
                nc.scalar.activation(out=inv[:], in_=var[:], func=AF.Ln, bias=epsc[:, 0:1])
                nc.scalar.activation(out=inv[:], in_=inv[:], func=AF.Exp, scale=-0.5)
                nc.vector.tensor_mul(out=sc[:], in0=gb_t[:, 0:1], in1=inv[:])
                nc.vector.tensor_mul(out=bi[:], in0=mu[:], in1=sc[:])
                nc.vector.tensor_sub(out=bi[:], in0=gb_t[:, 1:2], in1=bi[:])
                # relu pass + masked residual update (blockwise)
                for b in range(NB):
                    blk = slice(b * 896, (b + 1) * 896)
                    rt = bkp.tile([128, 896], FP32, tag="sqb")
                    nc.scalar.activation(out=rt[:], in_=xn_T[:, blk], func=AF.Relu,
                                         scale=sc[:, 0:1], bias=bi[:, 0:1])
                    if b == NB - 1:
                        nc.vector.tensor_mul(out=rt[:], in0=rt[:], in1=tmask_t[:])
                    nc.vector.tensor_add(out=h_T[:, blk], in0=h_T[:, blk],
                                         in1=rt[:])

            # ---------- pooling ----------
            pool_ps = []
            for _gh in range(GH):
                pacc = psb.tile([128, H], FP32, space="PSUM", tag="ps", name=f"pacc{_gh}")
                pool_ps.append(pacc)
            for t in range(NT):
                ohg_t = chp.tile([128, GP], FP32, tag="ohg")
                nc.sync.dma_start(out=ohg_t[:], in_=d_ohg[t])
                tps = psz.tile([128, 128], FP32, space="PSUM", tag="pz")
                nc.tensor.transpose(out=tps[:], in_=h_T[:, t * 128 : (t + 1) * 128],
                                    identity=ident[:])
                hn = edp.tile([128, 128], FP32, tag="hn")
                nc.vector.tensor_copy(out=hn[:], in_=tps[:])
                for gh in range(GH):
                    nc.tensor.matmul(out=pool_ps[gh][:],
                                     lhsT=ohg_t[:, gh * 128 : (gh + 1) * 128],
                                     rhs=hn[:], start=(t == 0), stop=(t == NT - 1),
                                     skip_group_check=True)
            for gh in range(GH):
                pt = tso.tile([128, H], FP32, tag="pt")
                nc.vector.tensor_copy(out=pt[:], in_=pool_ps[gh][:])
                nc.gpsimd.dma_start(out=pool_in[gh * 128 : (gh + 1) * 128, :],
                                    in_=pt[:])
            nc.gpsimd.collective_compute(
                "AllReduce", mybir.AluOpType.add, replica_groups=rg,
                ins=[pool_in[:]], outs=[pool_out[:]])

            # ---------- readout MLP (replicated) ----------
            invc_t = smp.tile([128, GH], FP32, tag="invc")
            nc.sync.dma_start(out=invc_t[:], in_=d_invc[:])
            pooled_T = pp.tile([128, GP], FP32)
            for gh in range(GH):
                q = edp.tile([128, H], FP32, tag="q")
                nc.gpsimd.dma_start(out=q[:], in_=pool_out[gh * 128 : (gh + 1) * 128, :])
                nc.vector.tensor_scalar_mul(out=q[:], in0=q[:],
                                            scalar1=invc_t[:, gh : gh + 1])
                tps = psz.tile([128, 128], FP32, space="PSUM", tag="pz")
                nc.tensor.transpose(out=tps[:], in_=q[:], identity=ident[:])
                nc.vector.tensor_copy(out=pooled_T[:, gh * 128 : (gh + 1) * 128],
                                      in_=tps[:])
            w1_t = smp.tile([H, 64], FP32, tag="w1")
            w2_t = smp.tile([64, 32], FP32, tag="w2")
            wout_t = smp.tile([32, 1], FP32, tag="wout")
            b1_t = smp.tile([64, 1], FP32, tag="b1")
            b2_t = smp.tile([32, 1], FP32, tag="b2")
            bout_t = smp.tile([1, 1], FP32, tag="bout")
            for tt, dd in ((w1_t, d_w1), (w2_t, d_w2), (wout_t, d_wout),
                           (b1_t, d_b1), (b2_t, d_b2), (bout_t, d_bout)):
                nc.sync.dma_start(out=tt[:], in_=dd[:])
            g1ps = psb.tile([64, GP], FP32, space="PSUM", tag="ps")
            nc.tensor.matmul(out=g1ps[:], lhsT=w1_t[:], rhs=pooled_T[:],
                             start=True, stop=True)
            g1 = edp.tile([64, GP], FP32, tag="g1")
            nc.scalar.activation(out=g1[:], in_=g1ps[:], func=AF.Relu,
                                 bias=b1_t[:, 0:1])
            g2ps = psb.tile([32, GP], FP32, space="PSUM", tag="ps")
            nc.tensor.matmul(out=g2ps[:], lhsT=w2_t[:], rhs=g1[:],
                             start=True, stop=True)
            g2 = edp.tile([32, GP], FP32, tag="g2")
            nc.scalar.activation(out=g2[:], in_=g2ps[:], func=AF.Relu,
                                 bias=b2_t[:, 0:1])
            ops = psb.tile([1, GP], FP32, space="PSUM", tag="ps")
            nc.tensor.matmul(out=ops[:], lhsT=wout_t[:], rhs=g2[:],
                             start=True, stop=True)
            ot = edp.tile([1, GP], FP32, tag="ot")
            nc.scalar.activation(out=ot[:], in_=ops[:], func=AF.Identity,
                                 bias=bout_t[:, 0:1])
            nc.sync.dma_start(out=d_out[None, :], in_=ot[:])

    # Pin all activations to the one table containing {exp, ln, relu,
    # identity, copy} so no ACT_TABLE_LOAD thrash occurs between Exp and Ln.
    import concourse.bacc as _bacc_mod
    _orig_gat = _bacc_mod.get_activation_tables

    def _pinned(arch):
        tabs = _orig_gat(arch)
        keep = "natural_log_exp_and_others"
        assert keep in tabs
        return {k: (v if k == keep else set()) for k, v in tabs.items()}

    _bacc_mod.get_activation_tables = _pinned
    try:
        nc.finalize()
    finally:
        _bacc_mod.get_activation_tables = _orig_gat
    return nc


def _run(cfg, inputs, trace=False):
    struct, in_maps = _prep(cfg, inputs["x"], inputs["edge_index"],
                            inputs["edge_attr"], inputs["batch"])
    wmap = _prep_weights(cfg, inputs["W_emb"], inputs["b_emb"], inputs["Wf"],
                         inputs["bf"], inputs["Ws"], inputs["bs"],
                         inputs["gamma"], inputs["beta"], inputs["W1"],
                         inputs["b1"], inputs["W2"], inputs["b2"],
                         inputs["W_out"], inputs["b_out"], inputs["batch"])
    for m in in_maps:
        m.update(wmap)
    nc = _build(cfg, struct)
    res = run_bass_kernel_spmd(nc, in_maps, list(range(NCORES)), trace=trace)
    out = res.results[0]["out"][: cfg.G].astype(np.float32)
    return out, res


def kernel(**inputs):
    x = np.asarray(inputs["x"])
    ei = np.asarray(inputs["edge_index"])
    batch = np.asarray(inputs["batch"])
    cfg = Cfg(N=x.shape[0], E=ei.shape[1], G=256)
    out, _ = _run(cfg, inputs)
    return out.astype(np.float32)


# revision 21
# speedup vs baseline: 2.3626x; 1.1046x over previous
"""CGCNN message-passing kernel for 8 Trainium2 NeuronCores.

Strategy (per core, per layer):
  - Nodes are sharded contiguously across cores (padded so the shard size is a
    multiple of lcm(112,128)=896). h lives feature-major ([128 feat, NPC nodes])
    in SBUF for the whole kernel.
  - Edges are sharded by dst shard, grouped into 112-node dst windows, and
    padded to 128-edge chunks on the host. All index-derived structures
    (onehots, gather indices, edge attrs) are precomputed on the host into
    dense input tensors; the device kernel is pure dense compute.
  - dst-side projections + edge-attr term: PE matmul with a host-built
    combined stationary [onehot(112); ea^T(8); 0(8)] against
    [Td_window; Wc; 0] — no gather needed (dst is shard-local).
  - src-side projections: one AllGather of the per-shard projection table
    T_src = h @ [Wf_b|Ws_b], then dma_gather (SWDGE) of 1KB rows per edge.
  - segment-sum: PE matmul m^T @ onehot accumulating in PSUM per window.
  - BatchNorm: feature-major reductions + a tiny AllReduce of [sum, sumsq].
  - Pooling: PE onehot matmul per node tile + AllReduce, then the small MLP
    replicated on every core.
"""

import math
import numpy as np

import concourse.bacc as bacc
import concourse.bass as bass
import concourse.tile as tile
import concourse.mybir as mybir
from concourse.bass_utils import run_bass_kernel_spmd
from concourse.masks import make_identity

FP32 = mybir.dt.float32
BF16 = mybir.dt.bfloat16
I16 = mybir.dt.int16

NCORES = 8
W = 112          # dst window (nodes); onehot rows 0..111, ea rows 112..119
SPLIT = 32768    # int16 dma_gather index limit -> lo/hi table split
CALLCAP = 8      # max 128-edge chunks per dma_gather call
EPS = 1e-5


class Cfg:
    def __init__(self, N, E, G, NF=16, EF=8, H=128, L=4):
        self.N, self.E, self.G, self.NF, self.EF, self.H, self.L = N, E, G, NF, EF, H, L
        per = math.ceil(N / NCORES)
        self.NPC = math.ceil(per / 896) * 896      # shard size (mult of 112 & 128)
        self.NP = self.NPC * NCORES                # padded node count
        self.NW = self.NPC // W                    # windows per shard
        self.NT = self.NPC // 128                  # 128-node tiles per shard
        self.GP = math.ceil(G / 128) * 128         # padded graph count
        self.GH = self.GP // 128                   # graph halves


def _prep(cfg, x, edge_index, edge_attr, batch):
    """Host-side preprocessing. Returns (structure, per-core input maps)."""
    N, E, G = cfg.N, cfg.E, cfg.G
    NPC, NW = cfg.NPC, cfg.NW
    src = np.asarray(edge_index[0], dtype=np.int64)
    dst = np.asarray(edge_index[1], dtype=np.int64)
    ea = np.asarray(edge_attr, dtype=np.float32)

    core = dst // NPC
    win = (dst % NPC) // W
    half = (src >= SPLIT).astype(np.int64)
    # group id per edge; sort once
    gid = (core * NW + win) * 2 + half
    order = np.argsort(gid * E + np.arange(E), kind="stable")  # stable group sort
    gid_s, src_s, dst_s, ea_s = gid[order], src[order], dst[order], ea[order]

    ngroups = NCORES * NW * 2
    cnt = np.bincount(gid_s, minlength=ngroups).reshape(NCORES, NW, 2)
    # global (uniform across cores) chunk counts per (win, half)
    nch = np.ceil(cnt.max(axis=0) / 128).astype(np.int64)  # [NW, 2]
    nch = ((nch + 1) // 2) * 2  # even so edge chunks can be processed in pairs
    # call split per (win, half)
    calls = []  # list of (w, half, chunk0, nchunks) in emission order
    chunk_of = []  # (w, half, k) for each global chunk index, in order
    for w in range(NW):
        for hf in range(2):
            n = int(nch[w, hf])
            k0 = 0
            while k0 < n:
                k = min(CALLCAP, n - k0)
                calls.append((w, hf, k0, k))
                k0 += k
            for k in range(n):
                chunk_of.append((w, hf, k))
    totch = len(chunk_of)

    group_start = np.zeros(ngroups + 1, dtype=np.int64)
    np.cumsum(np.bincount(gid_s, minlength=ngroups), out=group_start[1:])

    in_maps = []
    EFp = cfg.EF
    for c in range(NCORES):
        # ctab: cols 0..127 = comb ([onehot(112); ea(EF); 0]), cols 128..239 = oht
        import ml_dtypes
        ctab = np.zeros((totch, 128, 240), ml_dtypes.bfloat16)
        sidx_cols = []
        for gi, (w, hf, k) in enumerate(chunk_of):
            g = (c * NW + w) * 2 + hf
            s0, s1 = group_start[g], group_start[g + 1]
            e0 = s0 + k * 128
            e1 = min(s1, e0 + 128)
            if e1 > e0:
                n_e = e1 - e0
                dl = (dst_s[e0:e1] - (c * NPC + w * W)).astype(np.int64)
                ee = np.arange(n_e)
                ctab[gi, dl, ee] = 1.0
                ctab[gi, W : W + EFp, :n_e] = ea_s[e0:e1].T
                ctab[gi, ee, 128 + dl] = 1.0
        # gather indices per call
        for (w, hf, k0, k) in calls:
            g = (c * NW + w) * 2 + hf
            s0, s1 = group_start[g], group_start[g + 1]
            nidx = k * 128
            iv = np.zeros(nidx, np.int64)
            e0 = s0 + k0 * 128
            e1 = min(s1, e0 + nidx)
            if e1 > e0:
                iv[: e1 - e0] = src_s[e0:e1] - hf * SPLIT
            sidx_cols.append(iv.reshape(nidx // 16, 16).T.astype(np.int16))
        sidx = np.tile(np.concatenate(sidx_cols, axis=1), (8, 1))

        # node features, transposed + sharded
        xt = np.zeros((cfg.NF, NPC), np.float32)
        lo, hi = c * NPC, min((c + 1) * NPC, N)
        if hi > lo:
            xt[:, : hi - lo] = np.asarray(x[lo:hi], np.float32).T

        # pooling onehot [NT, 128, GP] and tail mask [128, 896]
        ohg = np.zeros((cfg.NT, 128, cfg.GP), np.float32)
        if hi > lo:
            nn = np.arange(lo, hi)
            b = np.asarray(batch[lo:hi], dtype=np.int64)
            ohg[(nn - lo) // 128, (nn - lo) % 128, b] = 1.0
        tmask = np.zeros((128, 896), np.float32)
        nreal = max(0, min(NPC, N - c * NPC))
        k = max(0, nreal - (NPC - 896))
        tmask[:, :k] = 1.0

        in_maps.append(
            {"ctab": ctab, "sidx": sidx, "xt": xt, "ohg": ohg, "tmask": tmask}
        )

    struct = {"nch": nch, "calls": calls, "chunk_of": chunk_of, "totch": totch,
              "sidx_cols": sum(cl[3] * 128 // 16 for cl in calls)}
    return struct, in_maps


def _prep_weights(cfg, W_emb, b_emb, Wf, bf, Ws, bs, gamma, beta, W1, b1, W2, b2,
                  W_out, b_out, batch):
    """Replicated weight tensors, packed for the device layouts."""
    L, H, EF, G = cfg.L, cfg.H, cfg.EF, cfg.G
    f32 = np.float32
    # The f-half (gate) is NEGATED everywhere so that one Exp(z) yields
    # exp(-mf) on the f-half and exp(ms) on the s-half.
    wsrc = np.stack([np.concatenate([-Wf[l][H : 2 * H], Ws[l][H : 2 * H]], 1)
                     for l in range(L)]).astype(f32)          # [L,128,256]
    wdst = np.stack([np.concatenate([-Wf[l][:H], Ws[l][:H]], 1)
                     for l in range(L)]).astype(f32)          # [L,128,256]
    wc = np.zeros((L, 16, 2 * H), f32)   # rows 8..15 stay zero (pad rows of rc)
    wc[:, :EF, :H] = -np.asarray(Wf, f32)[:, 2 * H :, :]
    wc[:, :EF, H:] = np.asarray(Ws, f32)[:, 2 * H :, :]       # [L,16,256]
    bias_b = np.zeros((L, 128, 2 * H), f32)
    bias_b[:, :W, :H] = -np.asarray(bf, f32)[:, None, :]
    bias_b[:, :W, H:] = np.asarray(bs, f32)[:, None, :]       # [L,128,256]
    gb = np.zeros((L, 128, 2), f32)
    gb[:, :H, 0] = np.asarray(gamma, f32)
    gb[:, :H, 1] = np.asarray(beta, f32)
    cnt = np.bincount(np.asarray(batch, np.int64), minlength=G).astype(f32)
    invc = np.zeros((128, cfg.GH), f32)
    ic = 1.0 / np.maximum(cnt, 1.0)
    icp = np.zeros(cfg.GP, f32)
    icp[:G] = ic
    invc[:, :] = icp.reshape(cfg.GH, 128).T
    return {
        "wemb": np.asarray(W_emb, f32),                        # [NF,128]
        "bemb": np.asarray(b_emb, f32).reshape(cfg.H, 1),
        "wsrc": wsrc, "wdst": wdst,
        "wc": __import__("ml_dtypes").bfloat16(wc), "bias_b": bias_b, "gb": gb,
        "invc": invc,
        "w1": np.asarray(W1, f32), "b1": np.asarray(b1, f32).reshape(-1, 1),
        "w2": np.asarray(W2, f32), "b2": np.asarray(b2, f32).reshape(-1, 1),
        "wout": np.asarray(W_out, f32), "bout": np.asarray(b_out, f32).reshape(1, 1),
    }


def _build(cfg, struct):
    """Trace the bass program. Returns nc."""
    NPC, NW, NT, L, H, NF = cfg.NPC, cfg.NW, cfg.NT, cfg.L, cfg.H, cfg.NF
    GP, GH = cfg.GP, cfg.GH
    nch, calls, chunk_of, totch = (struct["nch"], struct["calls"],
                                   struct["chunk_of"], struct["totch"])
    AF = mybir.ActivationFunctionType

    nc = bacc.Bacc("TRN2", target_bir_lowering=False, debug=False,
                   num_devices=NCORES)

    # ---- kernel I/O ----
    d_ctab = nc.declare_dram_parameter("ctab", [totch, 128, 240], BF16, isOutput=False)
    d_sidx = nc.declare_dram_parameter("sidx", [128, struct["sidx_cols"]], I16, isOutput=False)
    d_xt = nc.declare_dram_parameter("xt", [NF, NPC], FP32, isOutput=False)
    d_ohg = nc.declare_dram_parameter("ohg", [NT, 128, GP], FP32, isOutput=False)
    d_tmask = nc.declare_dram_parameter("tmask", [128, 896], FP32, isOutput=False)
    d_wemb = nc.declare_dram_parameter("wemb", [NF, H], FP32, isOutput=False)
    d_bemb = nc.declare_dram_parameter("bemb", [H, 1], FP32, isOutput=False)
    d_wsrc = nc.declare_dram_parameter("wsrc", [L, H, 2 * H], FP32, isOutput=False)
    d_wdst = nc.declare_dram_parameter("wdst", [L, H, 2 * H], FP32, isOutput=False)
    d_wc = nc.declare_dram_parameter("wc", [L, 16, 2 * H], BF16, isOutput=False)
    d_biasb = nc.declare_dram_parameter("bias_b", [L, 128, 2 * H], FP32, isOutput=False)
    d_gb = nc.declare_dram_parameter("gb", [L, 128, 2], FP32, isOutput=False)
    d_invc = nc.declare_dram_parameter("invc", [128, GH], FP32, isOutput=False)
    d_w1 = nc.declare_dram_parameter("w1", [H, 64], FP32, isOutput=False)
    d_b1 = nc.declare_dram_parameter("b1", [64, 1], FP32, isOutput=False)
    d_w2 = nc.declare_dram_parameter("w2", [64, 32], FP32, isOutput=False)
    d_b2 = nc.declare_dram_parameter("b2", [32, 1], FP32, isOutput=False)
    d_wout = nc.declare_dram_parameter("wout", [32, 1], FP32, isOutput=False)
    d_bout = nc.declare_dram_parameter("bout", [1, 1], FP32, isOutput=False)
    d_out = nc.declare_dram_parameter("out", [GP], FP32, isOutput=True)

    # ---- internal DRAM (collectives) ----
    ag_in = [nc.dram_tensor(f"ag_in{l}", [NPC, 2 * H], BF16) for l in range(L)]
    ts_full = [nc.dram_tensor(f"ts_full{l}", [cfg.NP, 2 * H], BF16,
                              addr_space="Shared") for l in range(L)]
    ar_in = [nc.dram_tensor(f"ar_in{l}", [128, 2], FP32) for l in range(L)]
    ar_out = [nc.dram_tensor(f"ar_out{l}", [128, 2], FP32, addr_space="Shared")
              for l in range(L)]
    pool_in = nc.dram_tensor("pool_in", [GP, H], FP32)
    pool_out = nc.dram_tensor("pool_out", [GP, H], FP32, addr_space="Shared")
    rg = [list(range(NCORES))]

    with tile.TileContext(nc) as tc:
        with (
            tc.tile_pool(name="persist", bufs=1) as pp,
            tc.tile_pool(name="lw", bufs=3) as lwp,       # per-layer weights
            tc.tile_pool(name="chunk", bufs=6) as chp,    # comb/oht tiles
            tc.tile_pool(name="gath", bufs=6) as gap,     # gather outputs
            tc.tile_pool(name="sidxp", bufs=6) as sxp,    # gather index tiles
            tc.tile_pool(name="edge", bufs=6) as edp,
            tc.tile_pool(name="blk", bufs=2) as bkp,     # z/s1/s2/m tiles
            tc.tile_pool(name="tsout", bufs=3) as tso,    # T_src build staging
            tc.tile_pool(name="small", bufs=2) as smp,
            tc.tile_pool(name="psz", bufs=3, space="PSUM") as psz,
            tc.tile_pool(name="psagg", bufs=2, space="PSUM") as psagg,
            tc.tile_pool(name="psb", bufs=2, space="PSUM") as psb,
        ):
            h_T = pp.tile([128, NPC], FP32)
            xn_T = pp.tile([128, NPC], FP32)
            ident = pp.tile([128, 128], FP32)
            tmask_t = pp.tile([128, 896], FP32)

            nc.sync.dma_start(out=tmask_t[:], in_=d_tmask[:])
            make_identity(nc, ident[:])
            epsc = pp.tile([128, 1], FP32)
            nc.vector.memset(epsc[:], EPS)

            # ---------- embedding: h = relu(x @ W_emb + b_emb) ----------
            wemb_t = smp.tile([NF, H], FP32)
            bemb_t = pp.tile([H, 1], FP32)
            nc.sync.dma_start(out=wemb_t[:], in_=d_wemb[:])
            nc.sync.dma_start(out=bemb_t[:], in_=d_bemb[:])
            EMBW = 448
            for s in range(NPC // EMBW):
                xs_t = edp.tile([NF, EMBW], FP32, tag="xs")
                nc.sync.dma_start(out=xs_t[:],
                                  in_=d_xt[:, s * EMBW : (s + 1) * EMBW])
                ps = psb.tile([128, EMBW], FP32, space="PSUM", tag="ps")
                nc.tensor.matmul(out=ps[:], lhsT=wemb_t[:], rhs=xs_t[:],
                                 start=True, stop=True)
                nc.scalar.activation(out=h_T[:, s * EMBW : (s + 1) * EMBW],
                                     in_=ps[:], func=AF.Relu, bias=bemb_t[:, 0:1])

            # ---------- layers ----------
            for l in range(L):
                wsrc_t = lwp.tile([H, 2 * H], FP32, tag="wsrc")
                wdst_t = lwp.tile([H, 2 * H], FP32, tag="wdst")
                wc_t = lwp.tile([16, 2 * H], BF16, tag="wc")
                biasb_t = lwp.tile([128, 2 * H], FP32, tag="biasb")
                gb_t = lwp.tile([128, 2], FP32, tag="gb")
                nc.sync.dma_start(out=wsrc_t[:], in_=d_wsrc[l])
                nc.sync.dma_start(out=wdst_t[:], in_=d_wdst[l])
                nc.sync.dma_start(out=wc_t[:], in_=d_wc[l])
                nc.sync.dma_start(out=biasb_t[:], in_=d_biasb[l])
                nc.sync.dma_start(out=gb_t[:], in_=d_gb[l])

                # ---- T_src shard build + AllGather ----
                for t in range(NT):
                    ps = psb.tile([128, 2 * H], FP32, space="PSUM", tag="ps")
                    nc.tensor.matmul(out=ps[:], lhsT=h_T[:, t * 128 : (t + 1) * 128],
                                     rhs=wsrc_t[:], start=True, stop=True)
                    st = tso.tile([128, 2 * H], BF16)
                    nc.vector.tensor_copy(out=st[:], in_=ps[:])
                    nc.gpsimd.dma_start(out=ag_in[l][t * 128 : (t + 1) * 128, :],
                                        in_=st[:])
                nc.gpsimd.collective_compute(
                    "AllGather", mybir.AluOpType.bypass, replica_groups=rg,
                    ins=[ag_in[l][:]], outs=[ts_full[l][:]])

                # ---- per-window: Td build, gathers, edge chunks ----
                calls_by_w = {}
                for cl in calls:
                    calls_by_w.setdefault(cl[0], []).append(cl)
                chunks_by_w = {}
                for gi, (w, hf, k) in enumerate(chunk_of):
                    chunks_by_w.setdefault(w, []).append((gi, hf, k))
                scol = 0
                for w in range(NW):
                    wchunks = chunks_by_w.get(w, [])
                    if not wchunks:
                        nc.vector.tensor_copy(out=xn_T[:, w * W : (w + 1) * W],
                                              in_=h_T[:, w * W : (w + 1) * W])
                        continue
                    # rhs_cat for this window: [Td+bias(112); Wc(8); junk(8)]
                    ps = psb.tile([128, 2 * H], FP32, space="PSUM", tag="ps")
                    nc.tensor.matmul(out=ps[:W, :], lhsT=h_T[:, w * W : (w + 1) * W],
                                     rhs=wdst_t[:], start=True, stop=True)
                    rc = lwp.tile([128, 2 * H], BF16, tag="rc")
                    nc.vector.tensor_add(out=rc[0:W, :], in0=ps[:W, :],
                                         in1=biasb_t[0:W, :])
                    nc.sync.dma_start(out=rc[W:128, :], in_=d_wc[l])
                    # src gathers for this window
                    gtiles = {}
                    for (_w, hf, k0, k) in calls_by_w.get(w, []):
                        nidx = k * 128
                        sx = sxp.tile([128, CALLCAP * 128 // 16], I16, tag="sx")
                        nc.sync.dma_start(out=sx[:, : nidx // 16],
                                          in_=d_sidx[:, scol : scol + nidx // 16])
                        scol += nidx // 16
                        g = gap.tile([128, CALLCAP, 2 * H], BF16, tag="gt")
                        base = ts_full[l][0 : min(SPLIT, cfg.NP), :] if hf == 0 \
                            else ts_full[l][SPLIT : cfg.NP, :]
                        nc.gpsimd.dma_gather(
                            out_ap=g[:, :k, :], in_ap=base,
                            idxs_ap=sx[:, : nidx // 16],
                            num_idxs=nidx, num_idxs_reg=nidx, elem_size=2 * H,
                            single_packet=False)
                        for kk in range(k):
                            gtiles[(hf, k0 + kk)] = (g, kk)
                    # edge chunks, processed in pairs (even count per window)
                    agg_ps = psagg.tile([128, W], FP32, space="PSUM", tag="agg")
                    npair = len(wchunks) // 2
                    for pi in range(npair):
                        gi, hf, k = wchunks[2 * pi]
                        ct = chp.tile([128, 2, 240], BF16, tag="ct")
                        nc.sync.dma_start(
                            out=ct[:],
                            in_=d_ctab[gi : gi + 2].rearrange("c p e -> p c e"))
                        pz = psz.tile([128, 2, 2 * H], FP32, space="PSUM", tag="pz")
                        for j in range(2):
                            nc.tensor.matmul(out=pz[:, j, :],
                                             lhsT=ct[:, j, 0:128], rhs=rc[:],
                                             start=True, stop=True)
                        g, kk = gtiles[(hf, k)]
                        z = edp.tile([128, 2, 2 * H], FP32, tag="z")
                        nc.vector.tensor_add(out=z[:], in0=pz[:],
                                             in1=g[:, kk : kk + 2, :])
                        # z = [-mf | ms] per chunk; e = exp(z); then
                        # m = ln(1+e_s) / (1+e_f)  (= softplus(ms)*sigmoid(mf))
                        e = edp.tile([128, 2, 2 * H], FP32, tag="e")
                        nc.scalar.activation(out=e[:], in_=z[:], func=AF.Exp)
                        t = edp.tile([128, 2, H], FP32, tag="t")
                        nc.scalar.activation(out=t[:], in_=e[:, :, H:], func=AF.Ln,
                                             bias=1.0)
                        d = edp.tile([128, 2, H], FP32, tag="d")
                        nc.vector.tensor_scalar_add(out=d[:], in0=e[:, :, :H],
                                                    scalar1=1.0)
                        r = edp.tile([128, 2, H], FP32, tag="r")
                        nc.vector.reciprocal_approx_fast(out=r[:], in_=d[:])
                        m = edp.tile([128, 2, H], BF16, tag="m")
                        nc.vector.tensor_mul(out=m[:], in0=t[:], in1=r[:])
                        for j in range(2):
                            nc.tensor.matmul(out=agg_ps[:], lhsT=m[:, j, :],
                                             rhs=ct[:, j, 128:240],
                                             start=(pi == 0 and j == 0),
                                             stop=(pi == npair - 1 and j == 1),
                                             skip_group_check=True)
                    nc.vector.tensor_add(out=xn_T[:, w * W : (w + 1) * W],
                                         in0=agg_ps[:],
                                         in1=h_T[:, w * W : (w + 1) * W])

                # ---- BatchNorm stats + AllReduce ----
                NB = NPC // 896
                part = smp.tile([128, 32], FP32, tag="part")
                for b in range(NB):
                    blk = xn_T[:, b * 896 : (b + 1) * 896]
                    nc.vector.reduce_sum(out=part[:, b : b + 1], in_=blk,
                                         axis=mybir.AxisListType.X)
                    sqb = bkp.tile([128, 896], FP32, tag="sqb")
                    nc.vector.tensor_mul(out=sqb[:], in0=blk, in1=blk)
                    nc.vector.reduce_sum(out=part[:, 16 + b : 17 + b], in_=sqb[:],
                                         axis=mybir.AxisListType.X)
                stats = smp.tile([128, 2], FP32, tag="stats")
                nc.vector.reduce_sum(out=stats[:, 0:1], in_=part[:, 0:NB],
                                     axis=mybir.AxisListType.X)
                nc.vector.reduce_sum(out=stats[:, 1:2], in_=part[:, 16 : 16 + NB],
                                     axis=mybir.AxisListType.X)
                nc.gpsimd.dma_start(out=ar_in[l][:], in_=stats[:])
                nc.gpsimd.collective_compute(
                    "AllReduce", mybir.AluOpType.add, replica_groups=rg,
                    ins=[ar_in[l][:]], outs=[ar_out[l][:]])
                st2 = smp.tile([128, 2], FP32, tag="st2")
                nc.gpsimd.dma_start(out=st2[:], in_=ar_out[l][:])
                mu = smp.tile([128, 1], FP32, tag="mu")
                var = smp.tile([128, 1], FP32, tag="var")
                inv = smp.tile([128, 1], FP32, tag="inv")
                sc = smp.tile([128, 1], FP32, tag="sc")
                bi = smp.tile([128, 1], FP32, tag="bi")
                rN = 1.0 / cfg.N
                nc.vector.tensor_scalar_mul(out=mu[:], in0=st2[:, 0:1], scalar1=rN)
                nc.vector.tensor_scalar_mul(out=var[:], in0=st2[:, 1:2], scalar1=rN)
                nc.vector.tensor_mul(out=inv[:], in0=mu[:], in1=mu[:])
                nc.vector.tensor_sub(out=var[:], in0=var[:], in1=inv[:])
                # 1/sqrt(var+eps) = exp(-0.5*ln(var+eps)) -- stays on Exp/Ln table
                nc.scalar.activation(out=inv[:], in_=var[:], func=AF.Ln, bias=epsc[:, 0:1])
                nc.scalar.activation(out=inv[:], in_=inv[:], func=AF.Exp, scale=-0.5)
                nc.vector.tensor_mul(out=sc[:], in0=gb_t[:, 0:1], in1=inv[:])
                nc.vector.tensor_mul(out=bi[:], in0=mu[:], in1=sc[:])
                nc.vector.tensor_sub(out=bi[:], in0=gb_t[:, 1:2], in1=bi[:])
                # relu pass + masked residual update (blockwise)
                for b in range(NB):
                    blk = slice(b * 896, (b + 1) * 896)
                    rt = bkp.tile([128, 896], FP32, tag="sqb")
                    nc.scalar.activation(out=rt[:], in_=xn_T[:, blk], func=AF.Relu,
                                         scale=sc[:, 0:1], bias=bi[:, 0:1])
                    if b == NB - 1:
                        nc.vector.tensor_mul(out=rt[:], in0=rt[:], in1=tmask_t[:])
                    nc.vector.tensor_add(out=h_T[:, blk], in0=h_T[:, blk],
                                         in1=rt[:])

            # ---------- pooling ----------
            pool_ps = []
            for _gh in range(GH):
                pacc = psb.tile([128, H], FP32, space="PSUM", tag="ps", name=f"pacc{_gh}")
                pool_ps.append(pacc)
            for t in range(NT):
                ohg_t = chp.tile([128, GP], FP32, tag="ohg")
                nc.sync.dma_start(out=ohg_t[:], in_=d_ohg[t])
                tps = psz.tile([128, 128], FP32, space="PSUM", tag="pz")
                nc.tensor.transpose(out=tps[:], in_=h_T[:, t * 128 : (t + 1) * 128],
                                    identity=ident[:])
                hn = edp.tile([128, 128], FP32, tag="hn")
                nc.vector.tensor_copy(out=hn[:], in_=tps[:])
                for gh in range(GH):
                    nc.tensor.matmul(out=pool_ps[gh][:],
                                     lhsT=ohg_t[:, gh * 128 : (gh + 1) * 128],
                                     rhs=hn[:], start=(t == 0), stop=(t == NT - 1),
                                     skip_group_check=True)
            for gh in range(GH):
                pt = tso.tile([128, H], FP32, tag="pt")
                nc.vector.tensor_copy(out=pt[:], in_=pool_ps[gh][:])
                nc.gpsimd.dma_start(out=pool_in[gh * 128 : (gh + 1) * 128, :],
                                    in_=pt[:])
            nc.gpsimd.collective_compute(
                "AllReduce", mybir.AluOpType.add, replica_groups=rg,
                ins=[pool_in[:]], outs=[pool_out[:]])

            # ---------- readout MLP (replicated) ----------
            invc_t = smp.tile([128, GH], FP32, tag="invc")
            nc.sync.dma_start(out=invc_t[:], in_=d_invc[:])
            pooled_T = pp.tile([128, GP], FP32)
            for gh in range(GH):
                q = edp.tile([128, H], FP32, tag="q")
                nc.gpsimd.dma_start(out=q[:], in_=pool_out[gh * 128 : (gh + 1) * 128, :])
                nc.vector.tensor_scalar_mul(out=q[:], in0=q[:],
                                            scalar1=invc_t[:, gh : gh + 1])
                tps = psz.tile([128, 128], FP32, space="PSUM", tag="pz")
                nc.tensor.transpose(out=tps[:], in_=q[:], identity=ident[:])
                nc.vector.tensor_copy(out=pooled_T[:, gh * 128 : (gh + 1) * 128],
                                      in_=tps[:])
            w1_t = smp.tile([H, 64], FP32, tag="w1")
            w2_t = smp.tile([64, 32], FP32, tag="w2")
            wout_t = smp.tile([32, 1], FP32, tag="wout")
            b1_t = smp.tile([64, 1], FP32, tag="b1")
            b2_t = smp.tile([32, 1], FP32, tag="b2")
            bout_t = smp.tile([1, 1], FP32, tag="bout")
            for tt, dd in ((w1_t, d_w1), (w2_t, d_w2), (wout_t, d_wout),
                           (b1_t, d_b1), (b2_t, d_b2), (bout_t, d_bout)):
                nc.sync.dma_start(out=tt[:], in_=dd[:])
            g1ps = psb.tile([64, GP], FP32, space="PSUM", tag="ps")
            nc.tensor.matmul(out=g1ps[:], lhsT=w1_t[:], rhs=pooled_T[:],
                             start=True, stop=True)
            g1 = edp.tile([64, GP], FP32, tag="g1")
            nc.scalar.activation(out=g1[:], in_=g1ps[:], func=AF.Relu,
                                 bias=b1_t[:, 0:1])
            g2ps = psb.tile([32, GP], FP32, space="PSUM", tag="ps")
            nc.tensor.matmul(out=g2ps[:], lhsT=w2_t[:], rhs=g1[:],
                             start=True, stop=True)
            g2 = edp.tile([32, GP], FP32, tag="g2")
            nc.scalar.activation(out=g2[:], in_=g2ps[:], func=AF.Relu,
                                 bias=b2_t[:, 0:1])
            ops = psb.tile([1, GP], FP32, space="PSUM", tag="ps")
            nc.tensor.matmul(out=ops[:], lhsT=wout_t[:], rhs=g2[:],
                             start=True, stop=True)
            ot = edp.tile([1, GP], FP32, tag="ot")
            nc.scalar.activation(out=ot[:], in_=ops[:], func=AF.Identity,
                                 bias=bout_t[:, 0:1])
            nc.sync.dma_start(out=d_out[None, :], in_=ot[:])

    # Pin all activations to the one table containing {exp, ln, relu,
    # identity, copy} so no ACT_TABLE_LOAD thrash occurs between Exp and Ln.
    import concourse.bacc as _bacc_mod
    _orig_gat = _bacc_mod.get_activation_tables

    def _pinned(arch):
        tabs = _orig_gat(arch)
        keep = "natural_log_exp_and_others"
        assert keep in tabs
        return {k: (v if k == keep else set()) for k, v in tabs.items()}

    _bacc_mod.get_activation_tables = _pinned
    try:
        nc.finalize()
    finally:
        _bacc_mod.get_activation_tables = _orig_gat
    return nc


def _run(cfg, inputs, trace=False):
    struct, in_maps = _prep(cfg, inputs["x"], inputs["edge_index"],
                            inputs["edge_attr"], inputs["batch"])
    wmap = _prep_weights(cfg, inputs["W_emb"], inputs["b_emb"], inputs["Wf"],
                         inputs["bf"], inputs["Ws"], inputs["bs"],
                         inputs["gamma"], inputs["beta"], inputs["W1"],
                         inputs["b1"], inputs["W2"], inputs["b2"],
                         inputs["W_out"], inputs["b_out"], inputs["batch"])
    for m in in_maps:
        m.update(wmap)
    nc = _build(cfg, struct)
    res = run_bass_kernel_spmd(nc, in_maps, list(range(NCORES)), trace=trace)
    out = res.results[0]["out"][: cfg.G].astype(np.float32)
    return out, res


def kernel(**inputs):
    x = np.asarray(inputs["x"])
    ei = np.asarray(inputs["edge_index"])
    batch = np.asarray(inputs["batch"])
    cfg = Cfg(N=x.shape[0], E=ei.shape[1], G=256)
    out, _ = _run(cfg, inputs)
    return out.astype(np.float32)


# revision 22
# speedup vs baseline: 2.4826x; 1.0508x over previous
"""CGCNN message-passing kernel for 8 Trainium2 NeuronCores.

Strategy (per core, per layer):
  - Nodes are sharded contiguously across cores (padded so the shard size is a
    multiple of lcm(112,128)=896). h lives feature-major ([128 feat, NPC nodes])
    in SBUF for the whole kernel.
  - Edges are sharded by dst shard, grouped into 112-node dst windows, and
    padded to 128-edge chunks on the host. All index-derived structures
    (onehots, gather indices, edge attrs) are precomputed on the host into
    dense input tensors; the device kernel is pure dense compute.
  - dst-side projections + edge-attr term: PE matmul with a host-built
    combined stationary [onehot(112); ea^T(8); 0(8)] against
    [Td_window; Wc; 0] — no gather needed (dst is shard-local).
  - src-side projections: one AllGather of the per-shard projection table
    T_src = h @ [Wf_b|Ws_b], then dma_gather (SWDGE) of 1KB rows per edge.
  - segment-sum: PE matmul m^T @ onehot accumulating in PSUM per window.
  - BatchNorm: feature-major reductions + a tiny AllReduce of [sum, sumsq].
  - Pooling: PE onehot matmul per node tile + AllReduce, then the small MLP
    replicated on every core.
"""

import math
import numpy as np

import concourse.bacc as bacc
import concourse.bass as bass
import concourse.tile as tile
import concourse.mybir as mybir
from concourse.bass_utils import run_bass_kernel_spmd
from concourse.masks import make_identity

FP32 = mybir.dt.float32
BF16 = mybir.dt.bfloat16
I16 = mybir.dt.int16

NCORES = 8
W = 112          # dst window (nodes); onehot rows 0..111, ea rows 112..119
SPLIT = 32768    # int16 dma_gather index limit -> lo/hi table split
CALLCAP = 8      # max 128-edge chunks per dma_gather call
EPS = 1e-5


class Cfg:
    def __init__(self, N, E, G, NF=16, EF=8, H=128, L=4):
        self.N, self.E, self.G, self.NF, self.EF, self.H, self.L = N, E, G, NF, EF, H, L
        per = math.ceil(N / NCORES)
        self.NPC = math.ceil(per / 896) * 896      # shard size (mult of 112 & 128)
        self.NP = self.NPC * NCORES                # padded node count
        self.NW = self.NPC // W                    # windows per shard
        self.NT = self.NPC // 128                  # 128-node tiles per shard
        self.GP = math.ceil(G / 128) * 128         # padded graph count
        self.GH = self.GP // 128                   # graph halves


def _prep(cfg, x, edge_index, edge_attr, batch):
    """Host-side preprocessing. Returns (structure, per-core input maps)."""
    N, E, G = cfg.N, cfg.E, cfg.G
    NPC, NW = cfg.NPC, cfg.NW
    src = np.asarray(edge_index[0], dtype=np.int64)
    dst = np.asarray(edge_index[1], dtype=np.int64)
    ea = np.asarray(edge_attr, dtype=np.float32)

    core = dst // NPC
    win = (dst % NPC) // W
    half = (src >= SPLIT).astype(np.int64)
    # group id per edge; sort once
    gid = (core * NW + win) * 2 + half
    order = np.argsort(gid * E + np.arange(E), kind="stable")  # stable group sort
    gid_s, src_s, dst_s, ea_s = gid[order], src[order], dst[order], ea[order]

    ngroups = NCORES * NW * 2
    cnt = np.bincount(gid_s, minlength=ngroups).reshape(NCORES, NW, 2)
    # global (uniform across cores) chunk counts per (win, half)
    nch = np.ceil(cnt.max(axis=0) / 128).astype(np.int64)  # [NW, 2]
    nch = ((nch + 1) // 2) * 2  # even so edge chunks can be processed in pairs
    # call split per (win, half)
    calls = []  # list of (w, half, chunk0, nchunks) in emission order
    chunk_of = []  # (w, half, k) for each global chunk index, in order
    for w in range(NW):
        for hf in range(2):
            n = int(nch[w, hf])
            k0 = 0
            while k0 < n:
                k = min(CALLCAP, n - k0)
                calls.append((w, hf, k0, k))
                k0 += k
            for k in range(n):
                chunk_of.append((w, hf, k))
    totch = len(chunk_of)

    group_start = np.zeros(ngroups + 1, dtype=np.int64)
    np.cumsum(np.bincount(gid_s, minlength=ngroups), out=group_start[1:])

    in_maps = []
    EFp = cfg.EF
    for c in range(NCORES):
        # ctab: cols 0..127 = comb ([onehot(112); ea(EF); 0]), cols 128..239 = oht
        import ml_dtypes
        ctab = np.zeros((totch, 128, 240), ml_dtypes.bfloat16)
        sidx_cols = []
        for gi, (w, hf, k) in enumerate(chunk_of):
            g = (c * NW + w) * 2 + hf
            s0, s1 = group_start[g], group_start[g + 1]
            e0 = s0 + k * 128
            e1 = min(s1, e0 + 128)
            if e1 > e0:
                n_e = e1 - e0
                dl = (dst_s[e0:e1] - (c * NPC + w * W)).astype(np.int64)
                ee = np.arange(n_e)
                ctab[gi, dl, ee] = 1.0
                ctab[gi, W : W + EFp, :n_e] = ea_s[e0:e1].T
                ctab[gi, ee, 128 + dl] = 1.0
        # gather indices per call
        for (w, hf, k0, k) in calls:
            g = (c * NW + w) * 2 + hf
            s0, s1 = group_start[g], group_start[g + 1]
            nidx = k * 128
            iv = np.zeros(nidx, np.int64)
            e0 = s0 + k0 * 128
            e1 = min(s1, e0 + nidx)
            if e1 > e0:
                iv[: e1 - e0] = src_s[e0:e1] - hf * SPLIT
            sidx_cols.append(iv.reshape(nidx // 16, 16).T.astype(np.int16))
        sidx = np.tile(np.concatenate(sidx_cols, axis=1), (8, 1))

        # node features, transposed + sharded
        xt = np.zeros((cfg.NF, NPC), np.float32)
        lo, hi = c * NPC, min((c + 1) * NPC, N)
        if hi > lo:
            xt[:, : hi - lo] = np.asarray(x[lo:hi], np.float32).T

        # pooling onehot [NT, 128, GP] and tail mask [128, 896]
        ohg = np.zeros((cfg.NT, 128, cfg.GP), np.float32)
        if hi > lo:
            nn = np.arange(lo, hi)
            b = np.asarray(batch[lo:hi], dtype=np.int64)
            ohg[(nn - lo) // 128, (nn - lo) % 128, b] = 1.0
        tmask = np.zeros((128, 896), np.float32)
        nreal = max(0, min(NPC, N - c * NPC))
        k = max(0, nreal - (NPC - 896))
        tmask[:, :k] = 1.0

        in_maps.append(
            {"ctab": ctab, "sidx": sidx, "xt": xt, "ohg": ohg, "tmask": tmask}
        )

    struct = {"nch": nch, "calls": calls, "chunk_of": chunk_of, "totch": totch,
              "sidx_cols": sum(cl[3] * 128 // 16 for cl in calls)}
    return struct, in_maps


def _prep_weights(cfg, W_emb, b_emb, Wf, bf, Ws, bs, gamma, beta, W1, b1, W2, b2,
                  W_out, b_out, batch):
    """Replicated weight tensors, packed for the device layouts."""
    L, H, EF, G = cfg.L, cfg.H, cfg.EF, cfg.G
    f32 = np.float32
    # The f-half (gate) is NEGATED everywhere so that one Exp(z) yields
    # exp(-mf) on the f-half and exp(ms) on the s-half.
    wsrc = np.stack([np.concatenate([-Wf[l][H : 2 * H], Ws[l][H : 2 * H]], 1)
                     for l in range(L)]).astype(f32)          # [L,128,256]
    wdst = np.stack([np.concatenate([-Wf[l][:H], Ws[l][:H]], 1)
                     for l in range(L)]).astype(f32)          # [L,128,256]
    wc = np.zeros((L, 16, 2 * H), f32)   # rows 8..15 stay zero (pad rows of rc)
    wc[:, :EF, :H] = -np.asarray(Wf, f32)[:, 2 * H :, :]
    wc[:, :EF, H:] = np.asarray(Ws, f32)[:, 2 * H :, :]       # [L,16,256]
    bias_b = np.zeros((L, 128, 2 * H), f32)
    bias_b[:, :W, :H] = -np.asarray(bf, f32)[:, None, :]
    bias_b[:, :W, H:] = np.asarray(bs, f32)[:, None, :]       # [L,128,256]
    gb = np.zeros((L, 128, 2), f32)
    gb[:, :H, 0] = np.asarray(gamma, f32)
    gb[:, :H, 1] = np.asarray(beta, f32)
    cnt = np.bincount(np.asarray(batch, np.int64), minlength=G).astype(f32)
    invc = np.zeros((128, cfg.GH), f32)
    ic = 1.0 / np.maximum(cnt, 1.0)
    icp = np.zeros(cfg.GP, f32)
    icp[:G] = ic
    invc[:, :] = icp.reshape(cfg.GH, 128).T
    return {
        "wemb": np.asarray(W_emb, f32),                        # [NF,128]
        "bemb": np.asarray(b_emb, f32).reshape(cfg.H, 1),
        "wsrc": wsrc, "wdst": wdst,
        "wc": __import__("ml_dtypes").bfloat16(wc), "bias_b": bias_b, "gb": gb,
        "invc": invc,
        "w1": np.asarray(W1, f32), "b1": np.asarray(b1, f32).reshape(-1, 1),
        "w2": np.asarray(W2, f32), "b2": np.asarray(b2, f32).reshape(-1, 1),
        "wout": np.asarray(W_out, f32), "bout": np.asarray(b_out, f32).reshape(1, 1),
    }


def _build(cfg, struct):
    """Trace the bass program. Returns nc."""
    NPC, NW, NT, L, H, NF = cfg.NPC, cfg.NW, cfg.NT, cfg.L, cfg.H, cfg.NF
    GP, GH = cfg.GP, cfg.GH
    nch, calls, chunk_of, totch = (struct["nch"], struct["calls"],
                                   struct["chunk_of"], struct["totch"])
    AF = mybir.ActivationFunctionType

    nc = bacc.Bacc("TRN2", target_bir_lowering=False, debug=False,
                   num_devices=NCORES)

    # ---- kernel I/O ----
    d_ctab = nc.declare_dram_parameter("ctab", [totch, 128, 240], BF16, isOutput=False)
    d_sidx = nc.declare_dram_parameter("sidx", [128, struct["sidx_cols"]], I16, isOutput=False)
    d_xt = nc.declare_dram_parameter("xt", [NF, NPC], FP32, isOutput=False)
    d_ohg = nc.declare_dram_parameter("ohg", [NT, 128, GP], FP32, isOutput=False)
    d_tmask = nc.declare_dram_parameter("tmask", [128, 896], FP32, isOutput=False)
    d_wemb = nc.declare_dram_parameter("wemb", [NF, H], FP32, isOutput=False)
    d_bemb = nc.declare_dram_parameter("bemb", [H, 1], FP32, isOutput=False)
    d_wsrc = nc.declare_dram_parameter("wsrc", [L, H, 2 * H], FP32, isOutput=False)
    d_wdst = nc.declare_dram_parameter("wdst", [L, H, 2 * H], FP32, isOutput=False)
    d_wc = nc.declare_dram_parameter("wc", [L, 16, 2 * H], BF16, isOutput=False)
    d_biasb = nc.declare_dram_parameter("bias_b", [L, 128, 2 * H], FP32, isOutput=False)
    d_gb = nc.declare_dram_parameter("gb", [L, 128, 2], FP32, isOutput=False)
    d_invc = nc.declare_dram_parameter("invc", [128, GH], FP32, isOutput=False)
    d_w1 = nc.declare_dram_parameter("w1", [H, 64], FP32, isOutput=False)
    d_b1 = nc.declare_dram_parameter("b1", [64, 1], FP32, isOutput=False)
    d_w2 = nc.declare_dram_parameter("w2", [64, 32], FP32, isOutput=False)
    d_b2 = nc.declare_dram_parameter("b2", [32, 1], FP32, isOutput=False)
    d_wout = nc.declare_dram_parameter("wout", [32, 1], FP32, isOutput=False)
    d_bout = nc.declare_dram_parameter("bout", [1, 1], FP32, isOutput=False)
    d_out = nc.declare_dram_parameter("out", [GP], FP32, isOutput=True)

    # ---- internal DRAM (collectives) ----
    ag_in = [nc.dram_tensor(f"ag_in{l}", [NPC, 2 * H], BF16) for l in range(L)]
    ts_full = [nc.dram_tensor(f"ts_full{l}", [cfg.NP, 2 * H], BF16,
                              addr_space="Shared") for l in range(L)]
    ar_in = [nc.dram_tensor(f"ar_in{l}", [128, 2], FP32) for l in range(L)]
    ar_out = [nc.dram_tensor(f"ar_out{l}", [128, 2], FP32, addr_space="Shared")
              for l in range(L)]
    pool_in = nc.dram_tensor("pool_in", [GP, H], FP32)
    pool_out = nc.dram_tensor("pool_out", [GP, H], FP32, addr_space="Shared")
    rg = [list(range(NCORES))]

    with tile.TileContext(nc) as tc:
        with (
            tc.tile_pool(name="persist", bufs=1) as pp,
            tc.tile_pool(name="lw", bufs=3) as lwp,       # per-layer weights
            tc.tile_pool(name="chunk", bufs=6) as chp,    # comb/oht tiles
            tc.tile_pool(name="gath", bufs=6) as gap,     # gather outputs
            tc.tile_pool(name="sidxp", bufs=6) as sxp,    # gather index tiles
            tc.tile_pool(name="edge", bufs=6) as edp,
            tc.tile_pool(name="blk", bufs=2) as bkp,     # z/s1/s2/m tiles
            tc.tile_pool(name="tsout", bufs=3) as tso,    # T_src build staging
            tc.tile_pool(name="small", bufs=2) as smp,
            tc.tile_pool(name="psz", bufs=3, space="PSUM") as psz,
            tc.tile_pool(name="psagg", bufs=2, space="PSUM") as psagg,
            tc.tile_pool(name="psb", bufs=2, space="PSUM") as psb,
        ):
            h_T = pp.tile([128, NPC], FP32)
            xn_T = pp.tile([128, NPC], FP32)
            ident = pp.tile([128, 128], FP32)
            tmask_t = pp.tile([128, 896], FP32)

            nc.sync.dma_start(out=tmask_t[:], in_=d_tmask[:])
            make_identity(nc, ident[:])
            epsc = pp.tile([128, 1], FP32)
            nc.vector.memset(epsc[:], EPS)
            identb = pp.tile([128, 128], BF16)
            make_identity(nc, identb[:])

            # ---------- embedding: h = relu(x @ W_emb + b_emb) ----------
            wemb_t = smp.tile([NF, H], FP32)
            bemb_t = pp.tile([H, 1], FP32)
            nc.sync.dma_start(out=wemb_t[:], in_=d_wemb[:])
            nc.sync.dma_start(out=bemb_t[:], in_=d_bemb[:])
            EMBW = 448
            for s in range(NPC // EMBW):
                xs_t = edp.tile([NF, EMBW], FP32, tag="xs")
                nc.sync.dma_start(out=xs_t[:],
                                  in_=d_xt[:, s * EMBW : (s + 1) * EMBW])
                ps = psb.tile([128, EMBW], FP32, space="PSUM", tag="ps")
                nc.tensor.matmul(out=ps[:], lhsT=wemb_t[:], rhs=xs_t[:],
                                 start=True, stop=True)
                nc.scalar.activation(out=h_T[:, s * EMBW : (s + 1) * EMBW],
                                     in_=ps[:], func=AF.Relu, bias=bemb_t[:, 0:1])

            # ---------- layers ----------
            for l in range(L):
                wsrc_t = lwp.tile([H, 2 * H], FP32, tag="wsrc")
                wdst_t = lwp.tile([H, 2 * H], FP32, tag="wdst")
                wc_t = lwp.tile([16, 2 * H], BF16, tag="wc")
                biasb_t = lwp.tile([128, 2 * H], FP32, tag="biasb")
                gb_t = lwp.tile([128, 2], FP32, tag="gb")
                nc.sync.dma_start(out=wsrc_t[:], in_=d_wsrc[l])
                nc.sync.dma_start(out=wdst_t[:], in_=d_wdst[l])
                nc.sync.dma_start(out=wc_t[:], in_=d_wc[l])
                nc.sync.dma_start(out=biasb_t[:], in_=d_biasb[l])
                nc.sync.dma_start(out=gb_t[:], in_=d_gb[l])

                # ---- T_src shard build + AllGather ----
                for t in range(NT):
                    ps = psb.tile([128, 2 * H], FP32, space="PSUM", tag="ps")
                    nc.tensor.matmul(out=ps[:], lhsT=h_T[:, t * 128 : (t + 1) * 128],
                                     rhs=wsrc_t[:], start=True, stop=True)
                    st = tso.tile([128, 2 * H], BF16)
                    nc.vector.tensor_copy(out=st[:], in_=ps[:])
                    nc.gpsimd.dma_start(out=ag_in[l][t * 128 : (t + 1) * 128, :],
                                        in_=st[:])
                nc.gpsimd.collective_compute(
                    "AllGather", mybir.AluOpType.bypass, replica_groups=rg,
                    ins=[ag_in[l][:]], outs=[ts_full[l][:]])

                # ---- per-window: Td build, gathers, edge chunks ----
                calls_by_w = {}
                for cl in calls:
                    calls_by_w.setdefault(cl[0], []).append(cl)
                chunks_by_w = {}
                for gi, (w, hf, k) in enumerate(chunk_of):
                    chunks_by_w.setdefault(w, []).append((gi, hf, k))
                scol = 0
                for w in range(NW):
                    wchunks = chunks_by_w.get(w, [])
                    if not wchunks:
                        nc.vector.tensor_copy(out=xn_T[:, w * W : (w + 1) * W],
                                              in_=h_T[:, w * W : (w + 1) * W])
                        continue
                    # rhs_cat for this window: [Td+bias(112); Wc(8); junk(8)]
                    ps = psb.tile([128, 2 * H], FP32, space="PSUM", tag="ps")
                    nc.tensor.matmul(out=ps[:W, :], lhsT=h_T[:, w * W : (w + 1) * W],
                                     rhs=wdst_t[:], start=True, stop=True)
                    rc = lwp.tile([128, 2 * H], BF16, tag="rc")
                    nc.vector.tensor_add(out=rc[0:W, :], in0=ps[:W, :],
                                         in1=biasb_t[0:W, :])
                    nc.sync.dma_start(out=rc[W:128, :], in_=d_wc[l])
                    # src gathers for this window
                    gtiles = {}
                    for (_w, hf, k0, k) in calls_by_w.get(w, []):
                        nidx = k * 128
                        sx = sxp.tile([128, CALLCAP * 128 // 16], I16, tag="sx")
                        nc.sync.dma_start(out=sx[:, : nidx // 16],
                                          in_=d_sidx[:, scol : scol + nidx // 16])
                        scol += nidx // 16
                        g = gap.tile([128, CALLCAP, 2 * H], BF16, tag="gt")
                        base = ts_full[l][0 : min(SPLIT, cfg.NP), :] if hf == 0 \
                            else ts_full[l][SPLIT : cfg.NP, :]
                        nc.gpsimd.dma_gather(
                            out_ap=g[:, :k, :], in_ap=base,
                            idxs_ap=sx[:, : nidx // 16],
                            num_idxs=nidx, num_idxs_reg=nidx, elem_size=2 * H,
                            single_packet=False)
                        for kk in range(k):
                            gtiles[(hf, k0 + kk)] = (g, kk)
                    # edge chunks, processed in pairs (even count per window)
                    agg_ps = psagg.tile([128, W], FP32, space="PSUM", tag="agg")
                    npair = len(wchunks) // 2
                    for pi in range(npair):
                        gi, hf, k = wchunks[2 * pi]
                        ct = chp.tile([128, 2, 240], BF16, tag="ct")
                        nc.sync.dma_start(
                            out=ct[:],
                            in_=d_ctab[gi : gi + 2].rearrange("c p e -> p c e"))
                        pz = psz.tile([128, 2, 2 * H], FP32, space="PSUM", tag="pz")
                        g, kk = gtiles[(hf, k)]
                        for j in range(2):
                            nc.tensor.matmul(out=pz[:, j, :],
                                             lhsT=ct[:, j, 0:128], rhs=rc[:],
                                             start=True, stop=False,
                                             skip_group_check=True)
                            nc.tensor.matmul(out=pz[:, j, :], lhsT=identb[:],
                                             rhs=g[:, kk + j, :],
                                             start=False, stop=True,
                                             skip_group_check=True)
                        # pz = [-mf | ms] per chunk; e = exp(pz); then
                        # m = ln(1+e_s) / (1+e_f)  (= softplus(ms)*sigmoid(mf))
                        e = edp.tile([128, 2, 2 * H], FP32, tag="e")
                        nc.scalar.activation(out=e[:], in_=pz[:], func=AF.Exp)
                        t = edp.tile([128, 2, H], FP32, tag="t")
                        nc.scalar.activation(out=t[:], in_=e[:, :, H:], func=AF.Ln,
                                             bias=1.0)
                        d = edp.tile([128, 2, H], FP32, tag="d")
                        nc.vector.tensor_scalar_add(out=d[:], in0=e[:, :, :H],
                                                    scalar1=1.0)
                        r = edp.tile([128, 2, H], FP32, tag="r")
                        nc.vector.reciprocal_approx_fast(out=r[:], in_=d[:])
                        m = edp.tile([128, 2, H], BF16, tag="m")
                        nc.vector.tensor_mul(out=m[:], in0=t[:], in1=r[:])
                        for j in range(2):
                            nc.tensor.matmul(out=agg_ps[:], lhsT=m[:, j, :],
                                             rhs=ct[:, j, 128:240],
                                             start=(pi == 0 and j == 0),
                                             stop=(pi == npair - 1 and j == 1),
                                             skip_group_check=True)
                    nc.vector.tensor_add(out=xn_T[:, w * W : (w + 1) * W],
                                         in0=agg_ps[:],
                                         in1=h_T[:, w * W : (w + 1) * W])

                # ---- BatchNorm stats + AllReduce ----
                NB = NPC // 896
                part = smp.tile([128, 32], FP32, tag="part")
                for b in range(NB):
                    blk = xn_T[:, b * 896 : (b + 1) * 896]
                    nc.vector.reduce_sum(out=part[:, b : b + 1], in_=blk,
                                         axis=mybir.AxisListType.X)
                    sqb = bkp.tile([128, 896], FP32, tag="sqb")
                    nc.vector.tensor_mul(out=sqb[:], in0=blk, in1=blk)
                    nc.vector.reduce_sum(out=part[:, 16 + b : 17 + b], in_=sqb[:],
                                         axis=mybir.AxisListType.X)
                stats = smp.tile([128, 2], FP32, tag="stats")
                nc.vector.reduce_sum(out=stats[:, 0:1], in_=part[:, 0:NB],
                                     axis=mybir.AxisListType.X)
                nc.vector.reduce_sum(out=stats[:, 1:2], in_=part[:, 16 : 16 + NB],
                                     axis=mybir.AxisListType.X)
                nc.gpsimd.dma_start(out=ar_in[l][:], in_=stats[:])
                nc.gpsimd.collective_compute(
                    "AllReduce", mybir.AluOpType.add, replica_groups=rg,
                    ins=[ar_in[l][:]], outs=[ar_out[l][:]])
                st2 = smp.tile([128, 2], FP32, tag="st2")
                nc.gpsimd.dma_start(out=st2[:], in_=ar_out[l][:])
                mu = smp.tile([128, 1], FP32, tag="mu")
                var = smp.tile([128, 1], FP32, tag="var")
                inv = smp.tile([128, 1], FP32, tag="inv")
                sc = smp.tile([128, 1], FP32, tag="sc")
                bi = smp.tile([128, 1], FP32, tag="bi")
                rN = 1.0 / cfg.N
                nc.vector.tensor_scalar_mul(out=mu[:], in0=st2[:, 0:1], scalar1=rN)
                nc.vector.tensor_scalar_mul(out=var[:], in0=st2[:, 1:2], scalar1=rN)
                nc.vector.tensor_mul(out=inv[:], in0=mu[:], in1=mu[:])
                nc.vector.tensor_sub(out=var[:], in0=var[:], in1=inv[:])
                # 1/sqrt(var+eps) = exp(-0.5*ln(var+eps)) -- stays on Exp/Ln table
                nc.scalar.activation(out=inv[:], in_=var[:], func=AF.Ln, bias=epsc[:, 0:1])
                nc.scalar.activation(out=inv[:], in_=inv[:], func=AF.Exp, scale=-0.5)
                nc.vector.tensor_mul(out=sc[:], in0=gb_t[:, 0:1], in1=inv[:])
                nc.vector.tensor_mul(out=bi[:], in0=mu[:], in1=sc[:])
                nc.vector.tensor_sub(out=bi[:], in0=gb_t[:, 1:2], in1=bi[:])
                # relu pass + masked residual update (blockwise)
                for b in range(NB):
                    blk = slice(b * 896, (b + 1) * 896)
                    rt = bkp.tile([128, 896], FP32, tag="sqb")
                    nc.scalar.activation(out=rt[:], in_=xn_T[:, blk], func=AF.Relu,
                                         scale=sc[:, 0:1], bias=bi[:, 0:1])
                    if b == NB - 1:
                        nc.vector.tensor_mul(out=rt[:], in0=rt[:], in1=tmask_t[:])
                    nc.vector.tensor_add(out=h_T[:, blk], in0=h_T[:, blk],
                                         in1=rt[:])

            # ---------- pooling ----------
            pool_ps = []
            for _gh in range(GH):
                pacc = psb.tile([128, H], FP32, space="PSUM", tag="ps", name=f"pacc{_gh}")
                pool_ps.append(pacc)
            for t in range(NT):
                ohg_t = chp.tile([128, GP], FP32, tag="ohg")
                nc.sync.dma_start(out=ohg_t[:], in_=d_ohg[t])
                tps = psz.tile([128, 128], FP32, space="PSUM", tag="pz")
                nc.tensor.transpose(out=tps[:], in_=h_T[:, t * 128 : (t + 1) * 128],
                                    identity=ident[:])
                hn = edp.tile([128, 128], FP32, tag="hn")
                nc.vector.tensor_copy(out=hn[:], in_=tps[:])
                for gh in range(GH):
                    nc.tensor.matmul(out=pool_ps[gh][:],
                                     lhsT=ohg_t[:, gh * 128 : (gh + 1) * 128],
                                     rhs=hn[:], start=(t == 0), stop=(t == NT - 1),
                                     skip_group_check=True)
            for gh in range(GH):
                pt = tso.tile([128, H], FP32, tag="pt")
                nc.vector.tensor_copy(out=pt[:], in_=pool_ps[gh][:])
                nc.gpsimd.dma_start(out=pool_in[gh * 128 : (gh + 1) * 128, :],
                                    in_=pt[:])
            nc.gpsimd.collective_compute(
                "AllReduce", mybir.AluOpType.add, replica_groups=rg,
                ins=[pool_in[:]], outs=[pool_out[:]])

            # ---------- readout MLP (replicated) ----------
            invc_t = smp.tile([128, GH], FP32, tag="invc")
            nc.sync.dma_start(out=invc_t[:], in_=d_invc[:])
            pooled_T = pp.tile([128, GP], FP32)
            for gh in range(GH):
                q = edp.tile([128, H], FP32, tag="q")
                nc.gpsimd.dma_start(out=q[:], in_=pool_out[gh * 128 : (gh + 1) * 128, :])
                nc.vector.tensor_scalar_mul(out=q[:], in0=q[:],
                                            scalar1=invc_t[:, gh : gh + 1])
                tps = psz.tile([128, 128], FP32, space="PSUM", tag="pz")
                nc.tensor.transpose(out=tps[:], in_=q[:], identity=ident[:])
                nc.vector.tensor_copy(out=pooled_T[:, gh * 128 : (gh + 1) * 128],
                                      in_=tps[:])
            w1_t = smp.tile([H, 64], FP32, tag="w1")
            w2_t = smp.tile([64, 32], FP32, tag="w2")
            wout_t = smp.tile([32, 1], FP32, tag="wout")
            b1_t = smp.tile([64, 1], FP32, tag="b1")
            b2_t = smp.tile([32, 1], FP32, tag="b2")
            bout_t = smp.tile([1, 1], FP32, tag="bout")
            for tt, dd in ((w1_t, d_w1), (w2_t, d_w2), (wout_t, d_wout),
                           (b1_t, d_b1), (b2_t, d_b2), (bout_t, d_bout)):
                nc.sync.dma_start(out=tt[:], in_=dd[:])
            g1ps = psb.tile([64, GP], FP32, space="PSUM", tag="ps")
            nc.tensor.matmul(out=g1ps[:], lhsT=w1_t[:], rhs=pooled_T[:],
                             start=True, stop=True)
            g1 = edp.tile([64, GP], FP32, tag="g1")
            nc.scalar.activation(out=g1[:], in_=g1ps[:], func=AF.Relu,
                                 bias=b1_t[:, 0:1])
            g2ps = psb.tile([32, GP], FP32, space="PSUM", tag="ps")
            nc.tensor.matmul(out=g2ps[:], lhsT=w2_t[:], rhs=g1[:],
                             start=True, stop=True)
            g2 = edp.tile([32, GP], FP32, tag="g2")
            nc.scalar.activation(out=g2[:], in_=g2ps[:], func=AF.Relu,
                                 bias=b2_t[:, 0:1])
            ops = psb.tile([1, GP], FP32, space="PSUM", tag="ps")
            nc.tensor.matmul(out=ops[:], lhsT=wout_t[:], rhs=g2[:],
                             start=True, stop=True)
            ot = edp.tile([1, GP], FP32, tag="ot")
            nc.scalar.activation(out=ot[:], in_=ops[:], func=AF.Identity,
                                 bias=bout_t[:, 0:1])
            nc.sync.dma_start(out=d_out[None, :], in_=ot[:])

    # Pin all activations to the one table containing {exp, ln, relu,
    # identity, copy} so no ACT_TABLE_LOAD thrash occurs between Exp and Ln.
    import concourse.bacc as _bacc_mod
    _orig_gat = _bacc_mod.get_activation_tables

    def _pinned(arch):
        tabs = _orig_gat(arch)
        keep = "natural_log_exp_and_others"
        assert keep in tabs
        return {k: (v if k == keep else set()) for k, v in tabs.items()}

    _bacc_mod.get_activation_tables = _pinned
    try:
        nc.finalize()
    finally:
        _bacc_mod.get_activation_tables = _orig_gat
    return nc


def _run(cfg, inputs, trace=False):
    struct, in_maps = _prep(cfg, inputs["x"], inputs["edge_index"],
                            inputs["edge_attr"], inputs["batch"])
    wmap = _prep_weights(cfg, inputs["W_emb"], inputs["b_emb"], inputs["Wf"],
                         inputs["bf"], inputs["Ws"], inputs["bs"],
                         inputs["gamma"], inputs["beta"], inputs["W1"],
                         inputs["b1"], inputs["W2"], inputs["b2"],
                         inputs["W_out"], inputs["b_out"], inputs["batch"])
    for m in in_maps:
        m.update(wmap)
    nc = _build(cfg, struct)
    res = run_bass_kernel_spmd(nc, in_maps, list(range(NCORES)), trace=trace)
    out = res.results[0]["out"][: cfg.G].astype(np.float32)
    return out, res


def kernel(**inputs):
    x = np.asarray(inputs["x"])
    ei = np.asarray(inputs["edge_index"])
    batch = np.asarray(inputs["batch"])
    cfg = Cfg(N=x.shape[0], E=ei.shape[1], G=256)
    out, _ = _run(cfg, inputs)
    return out.astype(np.float32)
